# revision 79
# baseline (speedup 1.0000x reference)
"""Trainium2 Bass kernel for nn_AttentionModel (pointer-network decode step).

Data-parallel over 8 NeuronCores: batch 512 -> 64 samples/core; weights
replicated.  Per core the kernel streams the per-sample K/V slabs from HBM
once in bf16 (~10.6 MB/group-phase, ~90 MB total) and computes:

  self-attn over (K_sa | k_sa) -> LN -> enc attention (masked) -> LN ->
  MLP -> LN -> single-head tanh-clipped pointer scores -> softmax weights.

v2 (bf16 streaming) layout notes:
  - all K/V streams, weights and matmul activations are bf16; PSUM, LN and
    softmax normalization stay fp32.  Streams are host-packed per group of
    4 samples into contiguous [128, ...] slabs so each dma_start moves ~1MB.
  - the on-device-computed new-token key k_sa is copied into column 256 of
    the streamed K tile, so one matmul accumulation produces all 257 scores.
  - scores/softmax use no max subtraction (scores are bounded by ~±7 for
    this model: q rows are LN outputs times 0.05-scale weights).
  - per-group attention outputs are folded into a single [64, 512] PSUM
    accumulator via a shifting block-diagonal ones matrix (Obig), replacing
    the per-group [4,512] fold + SBUF + DMA round trip.
  - the final single-head layer packs 8 samples per PSUM tile using a
    4x4 one-hot placement mask (pm4) so each 32-row band holds 4 samples.
"""

import numpy as np
import ml_dtypes
from contextlib import ExitStack

import concourse.bass as bass
import concourse.tile as tile
from concourse import bacc, mybir
from concourse.bass_utils import run_bass_kernel_spmd

f32 = mybir.dt.float32
bf16 = mybir.dt.bfloat16
fp8 = mybir.dt.float8e4
AF = mybir.ActivationFunctionType
ALU = mybir.AluOpType
AX = mybir.AxisListType

BF = ml_dtypes.bfloat16

P = 128          # SBUF partitions
NCORES = 8
B = 512          # full batch
BC = B // NCORES # batch per core (64)
D = 512          # model dim
H = 16           # heads
DH = 32          # head dim
NK = 251         # encoder keys (nb_nodes + 1)
NP = 256         # encoder keys padded to 256
T = 256          # self-attn cache length (new token appended on device)
NG = BC // 4     # sample groups of 4 (one [128, n] psum tile each)
NG2 = BC // 8    # stream pair-groups (two groups per ~2MB dma)
NGF = BC // 8    # final-layer groups of 8 samples
WNAMES = ["Wq_sa", "Wk_sa", "Wv_sa", "W0_sa", "Wq_a", "W0_a", "W1", "W2", "Wqf"]
# weight matmuls whose bias is applied on batch-major [64, 512] rows
BM_BIAS = {"Wv_sa", "W0_sa", "W0_a", "W2"}

_cache = {}


# ----------------------------------------------------------------------------
# program builder
# ----------------------------------------------------------------------------

def _build_program(flags):
    """flags = (use_bias tuple aligned with WNAMES, ln_affine tuple of 3)."""
    use_bias = dict(zip(WNAMES, flags[0]))
    ln_affine = flags[1]

    nc = bacc.Bacc("TRN2", target_bir_lowering=False, debug=False)

    def din(name, shape, dt=f32):
        return nc.dram_tensor(name, shape, dt, kind="ExternalInput").ap()

    hT_d = din("hT", [P, 4, BC], bf16)
    hbm_d = din("h_bm", [BC, D])
    ksa_d = din("ksa", [NG2, P, 2, 4, 4, T], bf16)
    vsa_d = din("vsa", [NG2, P, 2, 4, 2, D], bf16)
    ka0_d = din("ka0", [NG2, P, 2, 4, 4, NP], bf16)
    va0_d = din("va0", [NG2, P, 2, 4, 2, D], fp8)
    kaf_d = din("kaf", [NGF, P, 8, 4, NP], bf16)
    mask2_d = din("mask2", [NG, P, NP])
    maskF_d = din("maskF", [NGF, BC, NP])
    W_d = {n: din("W_" + n, [P, 4, D], bf16) for n in WNAMES}
    b_d = {n: din("b_" + n, [P, 4]) for n in WNAMES if use_bias[n]}
    bf_d = {n: din("bf_" + n, [1, D]) for n in WNAMES
            if use_bias[n] and n in BM_BIAS}
    if any(ln_affine):
        lnp_d = din("lnp", [6, D])
    identF_d = din("identF", [P, P])
    identB_d = din("identB", [P, P], bf16)
    obig_d = din("obig", [P, 124], bf16)
    bmask4_d = din("bmask4", [P, 4], bf16)
    bm4_d = din("bm4", [P, D])
    qm_d = din("qm", [P, 4, DH], bf16)
    pm4_d = din("pm4", [P, 8, 4], bf16)

    out_d = nc.dram_tensor("out", [BC, NK], f32, kind="ExternalOutput").ap()

    def bcast_row(ap2d, i, n):
        row = ap2d[i:i + 1, :]
        return bass.AP(tensor=row.tensor, offset=row.offset,
                       ap=[[0, n]] + list(row.ap)[1:])

    with tile.TileContext(nc) as tc, ExitStack() as ctx:
        consts = ctx.enter_context(tc.tile_pool(name="consts", bufs=1))
        pwts = ctx.enter_context(tc.tile_pool(name="wts", bufs=4))
        acts = ctx.enter_context(tc.tile_pool(name="acts", bufs=1))
        small = ctx.enter_context(tc.tile_pool(name="small", bufs=8))
        big_tmp = ctx.enter_context(tc.tile_pool(name="big_tmp", bufs=1))
        # stream pools: co-resident so cross-phase DMA prefetch never blocks
        pk1 = ctx.enter_context(tc.tile_pool(name="l1k", bufs=3))
        pv1 = ctx.enter_context(tc.tile_pool(name="l1v", bufs=2))
        pk2 = ctx.enter_context(tc.tile_pool(name="l2k", bufs=2))
        pv2 = ctx.enter_context(tc.tile_pool(name="l2v", bufs=2))
        pkf = ctx.enter_context(tc.tile_pool(name="fk", bufs=3))
        # attention scratch
        pqbd = ctx.enter_context(tc.tile_pool(name="qbd", bufs=2))
        pesc = ctx.enter_context(tc.tile_pool(name="esc", bufs=2))
        pwt = ctx.enter_context(tc.tile_pool(name="wt", bufs=2))
        pex = ctx.enter_context(tc.tile_pool(name="ex", bufs=2))
        pvr = ctx.enter_context(tc.tile_pool(name="vr", bufs=2))
        pfin = ctx.enter_context(tc.tile_pool(name="fin", bufs=2))
        pmsk = ctx.enter_context(tc.tile_pool(name="msk", bufs=2))
        # PSUM pools -- every distinct tag costs bufs x 1 bank; 8 banks total:
        #   ps_sc(1) + ps_wt(2) + ps_pt(2) + ps_at(1) + pj(2) = 8
        psc = ctx.enter_context(tc.tile_pool(name="psc", bufs=2, space="PSUM"))
        pswt = ctx.enter_context(tc.tile_pool(name="pswt", bufs=2, space="PSUM"))
        pspt = ctx.enter_context(tc.tile_pool(name="pspt", bufs=2, space="PSUM"))
        ppj = ctx.enter_context(tc.tile_pool(name="ppj", bufs=2, space="PSUM"))
        pacc = ppj

        # ------------------------------------------------------------------
        # constants / weights (x0/h first: the initial projections need them)
        # ------------------------------------------------------------------
        x0T = acts.tile([P, 4, BC], bf16, name="x0T", tag="x0T")
        nc.sync.dma_start(out=x0T, in_=hT_d)
        h_bm = acts.tile([BC, D], f32, name="h_bm", tag="h_bm")
        nc.sync.dma_start(out=h_bm, in_=hbm_d)
        # touch every ACT function we use so tables load during startup
        eps = consts.tile([P, 1], f32, name="eps", tag="eps")
        nc.vector.memset(eps, 1e-5)
        warm = consts.tile([P, 1], f32, name="warm", tag="warm")
        for fn in (AF.Exp, AF.Tanh, AF.Sqrt, AF.Identity, AF.Relu):
            nc.scalar.activation(warm, eps, fn, bias=0.0, scale=1.0)

        # pre-issue the first self-attn stream tiles and the first enc-attn
        # pair so they transfer while the startup consts/projections run
        pre_kt, pre_vt = {}, {}
        for g in range(2):
            pre_kt[g] = pk1.tile([P, 4, 4, 260], bf16, name="kt1", tag="kt1")
            nc.sync.dma_start(out=pre_kt[g][:, :, :, 0:T],
                              in_=ksa_d[g // 2][:, g % 2])
            pre_vt[g] = pv1.tile([P, 4, 2, D], bf16, name="vt1", tag="vt1")
            nc.sync.dma_start(out=pre_vt[g], in_=vsa_d[g // 2][:, g % 2])
        pre_kt2 = pk2.tile([P, 2, 4, 4, NP], bf16, name="kt2", tag="kt2")
        nc.sync.dma_start(out=pre_kt2, in_=ka0_d[0])
        pre_vt2 = pv2.tile([P, 2, 4, 2, D], fp8, name="vt2", tag="vt2")
        nc.sync.dma_start(out=pre_vt2, in_=va0_d[0])

        identF = consts.tile([P, P], f32, name="identF", tag="identF")
        nc.sync.dma_start(out=identF, in_=identF_d)
        identB = consts.tile([P, P], bf16, name="identB", tag="identB")
        nc.sync.dma_start(out=identB, in_=identB_d)
        obig = consts.tile([P, 124], bf16, name="obig", tag="obig")
        nc.sync.dma_start(out=obig, in_=obig_d)
        bmask4 = consts.tile([P, 4], bf16, name="bmask4", tag="bmask4")
        nc.sync.dma_start(out=bmask4, in_=bmask4_d)
        bm4 = consts.tile([P, D], f32, name="bm4", tag="bm4")
        nc.sync.dma_start(out=bm4, in_=bm4_d)
        qm = consts.tile([P, 4, DH], bf16, name="qm", tag="qm")
        nc.sync.dma_start(out=qm, in_=qm_d)
        pm4 = consts.tile([P, 8, 4], bf16, name="pm4", tag="pm4")
        nc.sync.dma_start(out=pm4, in_=pm4_d)

        Wt, bt, bft = {}, {}, {}
        for n in WNAMES:
            Wt[n] = pwts.tile([P, 4, D], bf16, name="W_" + n, tag="W")
            nc.gpsimd.dma_start(out=Wt[n], in_=W_d[n])
            if use_bias[n]:
                bt[n] = consts.tile([P, 4], f32, name="b_" + n, tag="b_" + n)
                nc.sync.dma_start(out=bt[n], in_=b_d[n])
                if n in BM_BIAS:
                    bft[n] = consts.tile([BC, D], f32, name="bf_" + n, tag="bf_" + n)
                    nc.sync.dma_start(out=bft[n], in_=bcast_row(bf_d[n], 0, BC))

        lng, lnb = [None] * 3, [None] * 3
        for i in range(3):
            if ln_affine[i]:
                lng[i] = consts.tile([BC, D], f32, name=f"lng{i}", tag=f"lng{i}")
                nc.sync.dma_start(out=lng[i], in_=bcast_row(lnp_d, 2 * i, BC))
                lnb[i] = consts.tile([BC, D], f32, name=f"lnb{i}", tag=f"lnb{i}")
                nc.sync.dma_start(out=lnb[i], in_=bcast_row(lnp_d, 2 * i + 1, BC))

        # ------------------------------------------------------------------
        # helpers
        # ------------------------------------------------------------------
        def proj_dmajor(dst, wname, src_T, relu=False):
            """dst[:, mc, :] (d-major [128, 4, 64] bf16) = act(x @ W + b)."""
            for mc in range(4):
                ps = ppj.tile([P, BC], f32, name="pp_" + wname, tag="pj")
                for kc in range(4):
                    nc.tensor.matmul(
                        ps,
                        lhsT=Wt[wname][:, kc, mc * P:(mc + 1) * P],
                        rhs=src_T[:, kc, :],
                        start=(kc == 0), stop=(kc == 3),
                    )
                bias = bt[wname][:, mc:mc + 1] if use_bias[wname] else 0.0
                func = AF.Relu if relu else AF.Identity
                nc.scalar.activation(dst[:, mc, :], ps, func, bias=bias, scale=1.0)

        def mm_batchmajor(ps, src_T, wname):
            """ps [64, 512] = x @ W   (lhsT = x^T chunks, W as moving)."""
            for kc in range(4):
                nc.tensor.matmul(
                    ps,
                    lhsT=src_T[:, kc, :],
                    rhs=Wt[wname][:, kc, :],
                    start=(kc == 0), stop=(kc == 3),
                )

        def transpose_bm(dst_T, src_bm):
            """[64, 512] f32 batch-major -> d-major [128, 4, 64] bf16."""
            for c in range(4):
                ps = pswt.tile([P, BC], f32, name="ptr", tag="ps_wt")
                nc.tensor.transpose(ps, src_bm[:, c * P:(c + 1) * P],
                                    identF[0:BC, 0:BC])
                nc.vector.tensor_copy(dst_T[:, c, :], ps)

        def layer_norm(dst_bm, t_bm, idx):
            stats = small.tile([BC, 6], f32, name="stats", tag="stats")
            nc.vector.bn_stats(stats, t_bm)
            mv = small.tile([BC, 2], f32, name="mv", tag="mv")
            nc.vector.bn_aggr(mv, stats)
            sd = small.tile([BC, 1], f32, name="sd", tag="sd")
            nc.scalar.activation(sd, mv[:, 1:2], AF.Sqrt, bias=eps[0:BC], scale=1.0)
            rstd = small.tile([BC, 1], f32, name="rstd", tag="rstd")
            nc.vector.reciprocal(rstd, sd)
            nmr = small.tile([BC, 1], f32, name="nmr", tag="nmr")
            nc.vector.scalar_tensor_tensor(out=nmr, in0=mv[:, 0:1], scalar=-1.0,
                                           in1=rstd, op0=ALU.mult, op1=ALU.mult)
            if ln_affine[idx]:
                xn = big_tmp.tile([BC, D], f32, name="xn", tag="xn")
                nc.scalar.activation(xn, t_bm, AF.Identity, bias=nmr, scale=rstd)
                nc.vector.tensor_mul(xn, xn, lng[idx])
                nc.vector.tensor_add(dst_bm, xn, lnb[idx])
            else:
                nc.scalar.activation(dst_bm, t_bm, AF.Identity, bias=nmr, scale=rstd)

        def residual_ln(dst_bm, dst_T, src_T, wname, x_prev_bm, idx):
            """dst = LN(x_prev + src @ W + b); also produce d-major dst_T."""
            ps = ppj.tile([BC, D], f32, name="pr_" + wname, tag="pj")
            mm_batchmajor(ps, src_T, wname)
            t_bm = big_tmp.tile([BC, D], f32, name="t_bm", tag="t_bm")
            nc.vector.tensor_add(t_bm, ps, x_prev_bm)
            if use_bias[wname]:
                nc.vector.tensor_add(t_bm, t_bm, bft[wname])
            layer_norm(dst_bm, t_bm, idx)
            transpose_bm(dst_T, dst_bm)

        def build_qbd(q_T, g):
            """[128, 4(j), 4(c), 32] bf16: sample 4g+j's q placed per-head."""
            qbd = pqbd.tile([P, 4, 4, DH], bf16, name="qbd", tag="qbd")
            in0 = q_T[:, :, 4 * g:4 * g + 4].transpose([0, 2, 1]) \
                .unsqueeze(3).broadcast_to([P, 4, 4, DH])
            in1 = qm.unsqueeze(1).broadcast_to([P, 4, 4, DH])
            nc.vector.tensor_mul(qbd, in0, in1)
            return qbd

        # ------------------------------------------------------------------
        # projections from x0 = h_t
        # ------------------------------------------------------------------
        q_saT = acts.tile([P, 4, BC], bf16, name="q_saT", tag="q_saT")
        proj_dmajor(q_saT, "Wq_sa", x0T)
        k_saT = acts.tile([P, 4, BC], bf16, name="k_saT", tag="k_saT")
        proj_dmajor(k_saT, "Wk_sa", x0T)

        v_bmb = acts.tile([BC, D], bf16, name="v_bmb", tag="v_bmb")
        psv = ppj.tile([BC, D], f32, name="psv", tag="pj")
        mm_batchmajor(psv, x0T, "Wv_sa")
        if use_bias["Wv_sa"]:
            tv = big_tmp.tile([BC, D], f32, name="tv", tag="tv")
            nc.vector.tensor_add(tv, psv, bft["Wv_sa"])
            nc.vector.tensor_copy(v_bmb, tv)
        else:
            nc.scalar.copy(v_bmb, psv)

        # ------------------------------------------------------------------
        # layer 1: self-attention over (K_sa | k_sa)
        # ------------------------------------------------------------------
        wcols = T + 1
        ps_at1 = pacc.tile([BC, D], f32, name="ps_at1", tag="pj")
        for g in range(NG):
            if g in pre_kt:
                kt, vt = pre_kt[g], pre_vt[g]
            else:
                # 260-col rows keep each (j, c) row 8-byte aligned in SBUF
                kt = pk1.tile([P, 4, 4, 260], bf16, name="kt1", tag="kt1")
                nc.sync.dma_start(out=kt[:, :, :, 0:T],
                                  in_=ksa_d[g // 2][:, g % 2])
                vt = pv1.tile([P, 4, 2, D], bf16, name="vt1", tag="vt1")
                nc.sync.dma_start(out=vt, in_=vsa_d[g // 2][:, g % 2])
            # new-token key column (k_saT is [P, 4(c), BC])
            nc.vector.tensor_copy(
                kt[:, :, :, T:T + 1],
                k_saT[:, :, 4 * g:4 * g + 4].transpose([0, 2, 1]).unsqueeze(3))
            qbd = build_qbd(q_saT, g)
            vr4 = pvr.tile([4, D], bf16, name="vr4", tag="vr")
            nc.gpsimd.dma_start(out=vr4, in_=v_bmb[4 * g:4 * g + 4, :])

            ps_sc = psc.tile([P, wcols], f32, name="ps_sc", tag="ps_sc")
            for j in range(4):
                for c in range(4):
                    nc.tensor.matmul(
                        ps_sc[32 * j:32 * j + 32, :],
                        lhsT=qbd[:, j, c, :],
                        rhs=kt[:, j, c, 0:wcols],
                        start=(c == 0), stop=(c == 3),
                        tile_position=(0, 32 * j))
            esc = pesc.tile([P, wcols], bf16, name="esc", tag="esc")
            sumexp = small.tile([P, 1], f32, name="sumexp", tag="sumexp")
            nc.scalar.activation(esc, ps_sc, AF.Exp, bias=0.0, scale=1.0,
                                 accum_out=sumexp)
            recip = small.tile([P, 1], f32, name="recip", tag="recip")
            nc.vector.reciprocal(recip, sumexp)
            # new-token softmax weights, banded: escb4[p, r] =
            # esc[p, 256] * (p//32 == r); transposed it becomes the K=4
            # block-diagonal lhsT for the new-token value product.
            escb4 = pesc.tile([P, 4], bf16, name="escb4", tag="escb4")
            nc.vector.tensor_mul(escb4, esc[:, T:T + 1].broadcast_to([P, 4]),
                                 bmask4)

            ps_wt = pswt.tile([P, 3, P], bf16, name="ps_wt", tag="ps_wt")
            wt = pwt.tile([P, 3, P], bf16, name="wt", tag="wt")
            for c, cw, src in [(0, P, esc[:, 0:P]), (1, P, esc[:, P:2 * P]),
                               (2, 4, escb4)]:
                nc.tensor.transpose(ps_wt[0:cw, c, :], src, identB)
                nc.vector.tensor_copy(wt[0:cw, c, :], ps_wt[0:cw, c, :])

            ps_pt = pspt.tile([P, D], f32, name="ps_pt", tag="ps_pt")
            for j in range(4):
                for kc in range(2):
                    nc.tensor.matmul(
                        ps_pt[32 * j:32 * j + 32, :],
                        lhsT=wt[:, kc, 32 * j:32 * j + 32],
                        rhs=vt[:, j, kc, :],
                        start=(kc == 0), stop=False,
                        tile_position=(0, 32 * j), skip_group_check=True)
            # new-token contribution last, so the band matmuls above never
            # wait on the (small, late) vr4 staging dma
            nc.tensor.matmul(ps_pt, lhsT=wt[0:4, 2, :], rhs=vr4,
                             start=False, stop=True, skip_group_check=True)
            ex = pex.tile([P, D], bf16, name="ex", tag="ex")
            nc.vector.scalar_tensor_tensor(
                out=ex, in0=ps_pt, scalar=recip, in1=bm4,
                op0=ALU.mult, op1=ALU.mult)
            nc.tensor.matmul(ps_at1, lhsT=obig[:, 60 - 4 * g:124 - 4 * g],
                             rhs=ex, start=(g == 0), stop=(g == NG - 1))

        attn1 = acts.tile([BC, D], f32, name="attn1", tag="attn1")
        nc.scalar.copy(attn1, ps_at1)
        x1_bm = acts.tile([BC, D], f32, name="x1_bm", tag="x1_bm")
        x1T = acts.tile([P, 4, BC], bf16, name="x1T", tag="x1T")
        attn1T = acts.tile([P, 4, BC], bf16, name="attn1T", tag="attn1T")
        transpose_bm(attn1T, attn1)
        residual_ln(x1_bm, x1T, attn1T, "W0_sa", h_bm, 0)

        # ------------------------------------------------------------------
        # layer 2: encoder attention (masked, padded keys)
        # ------------------------------------------------------------------
        q_aT = acts.tile([P, 4, BC], bf16, name="q_aT", tag="q_aT")
        proj_dmajor(q_aT, "Wq_a", x1T)

        ps_at2 = pacc.tile([BC, D], f32, name="ps_at2", tag="pj")
        for G2 in range(NG2):
            if G2 == 0:
                kt, vt = pre_kt2, pre_vt2
            else:
                kt = pk2.tile([P, 2, 4, 4, NP], bf16, name="kt2", tag="kt2")
                nc.sync.dma_start(out=kt, in_=ka0_d[G2])
                vt = pv2.tile([P, 2, 4, 2, D], fp8, name="vt2", tag="vt2")
                nc.sync.dma_start(out=vt, in_=va0_d[G2])
            for u in range(2):
                g = 2 * G2 + u
                qbd = build_qbd(q_aT, g)

                mt = pmsk.tile([P, NP], f32, name="mt", tag="m2")
                nc.sync.dma_start(out=mt, in_=mask2_d[g])

                ps_sc = psc.tile([P, NP], f32, name="ps_sc2", tag="ps_sc")
                for j in range(4):
                    for c in range(4):
                        nc.tensor.matmul(
                            ps_sc[32 * j:32 * j + 32, :],
                            lhsT=qbd[:, j, c, :],
                            rhs=kt[:, u, j, c, :],
                            start=(c == 0), stop=(c == 3),
                            tile_position=(0, 32 * j))
                nc.vector.tensor_add(ps_sc, ps_sc, mt)
                esc = pesc.tile([P, NP], bf16, name="esc2", tag="esc")
                sumexp = small.tile([P, 1], f32, name="sumexp2", tag="sumexp")
                nc.scalar.activation(esc, ps_sc, AF.Exp, bias=0.0, scale=1.0,
                                     accum_out=sumexp)
                recip = small.tile([P, 1], f32, name="recip2", tag="recip")
                nc.vector.reciprocal(recip, sumexp)

                ps_wt = pswt.tile([P, 2, P], bf16, name="ps_wt2", tag="ps_wt")
                wt = pwt.tile([P, 2, P], bf16, name="wt2", tag="wt")
                for c in range(2):
                    nc.tensor.transpose(ps_wt[:, c, :],
                                        esc[:, c * P:(c + 1) * P], identB)
                    nc.vector.tensor_copy(wt[:, c, :], ps_wt[:, c, :])

                ps_pt = pspt.tile([P, D], f32, name="ps_pt2", tag="ps_pt")
                for j in range(4):
                    for kc in range(2):
                        nc.tensor.matmul(
                            ps_pt[32 * j:32 * j + 32, :],
                            lhsT=wt[:, kc, 32 * j:32 * j + 32],
                            rhs=vt[:, u, j, kc, :],
                            start=(kc == 0), stop=(kc == 1),
                            tile_position=(0, 32 * j))
                ex = pex.tile([P, D], bf16, name="ex2", tag="ex")
                nc.vector.scalar_tensor_tensor(
                    out=ex, in0=ps_pt, scalar=recip, in1=bm4,
                    op0=ALU.mult, op1=ALU.mult)
                nc.tensor.matmul(ps_at2, lhsT=obig[:, 60 - 4 * g:124 - 4 * g],
                                 rhs=ex, start=(g == 0), stop=(g == NG - 1))

        attn2 = acts.tile([BC, D], f32, name="attn2", tag="attn2")
        nc.scalar.copy(attn2, ps_at2)
        x2_bm = acts.tile([BC, D], f32, name="x2_bm", tag="x2_bm")
        x2T = acts.tile([P, 4, BC], bf16, name="x2T", tag="x2T")
        attn2T = acts.tile([P, 4, BC], bf16, name="attn2T", tag="attn2T")
        transpose_bm(attn2T, attn2)
        residual_ln(x2_bm, x2T, attn2T, "W0_a", x1_bm, 1)

        # ------------------------------------------------------------------
        # MLP
        # ------------------------------------------------------------------
        h1T = acts.tile([P, 4, BC], bf16, name="h1T", tag="h1T")
        proj_dmajor(h1T, "W1", x2T, relu=True)
        x3_bm = acts.tile([BC, D], f32, name="x3_bm", tag="x3_bm")
        x3T = acts.tile([P, 4, BC], bf16, name="x3T", tag="x3T")
        residual_ln(x3_bm, x3T, h1T, "W2", x2_bm, 2)

        qfT = acts.tile([P, 4, BC], bf16, name="qfT", tag="qfT")
        proj_dmajor(qfT, "Wqf", x3T)

        # ------------------------------------------------------------------
        # final pointer scores: w = softmax(10*tanh(qf.K/sqrt(D)) + mask)
        # 8 samples per group: rows 32*q4 + r  (q4 in 0..2, r in 0..4)
        # ------------------------------------------------------------------
        for G in range(NGF):
            kf = pkf.tile([P, 8, 4, NP], bf16, name="kf", tag="kf")
            nc.gpsimd.dma_start(out=kf, in_=kaf_d[G])
            # qfb[p, c, s, r] = qfT[p, c, 8G+s] * (s%4 == r)
            qfb = pqbd.tile([P, 4, 8, 4], bf16, name="qfb", tag="qfb")
            in0 = qfT[:, :, 8 * G:8 * G + 8].unsqueeze(3) \
                .broadcast_to([P, 4, 8, 4])
            in1 = pm4.unsqueeze(1).broadcast_to([P, 4, 8, 4])
            nc.vector.tensor_mul(qfb, in0, in1)

            ps_f = psc.tile([BC, NP], f32, name="ps_f", tag="ps_sc")
            nc.vector.memset(ps_f, 0.0)
            for q4 in range(2):
                for r in range(4):
                    for c in range(4):
                        nc.tensor.matmul(
                            ps_f[32 * q4:32 * q4 + 4, :],
                            lhsT=qfb[:, c, 4 * q4 + r, :],
                            rhs=kf[:, 4 * q4 + r, c, :],
                            start=(r == 0 and c == 0), stop=(r == 3 and c == 3),
                            tile_position=(0, 32 * q4))
            mf = pmsk.tile([BC, NP], f32, name="mf", tag="mf")
            nc.sync.dma_start(out=mf, in_=maskF_d[G])
            t1 = pfin.tile([BC, NP], f32, name="t1", tag="t1")
            nc.scalar.activation(t1, ps_f, AF.Tanh, bias=0.0, scale=1.0)
            t2 = pfin.tile([BC, NP], f32, name="t2", tag="t2")
            nc.vector.scalar_tensor_tensor(out=t2, in0=t1, scalar=10.0,
                                           in1=mf,
                                           op0=ALU.mult, op1=ALU.add)
            e = pfin.tile([BC, NP], f32, name="e", tag="e")
            fsum = small.tile([BC, 1], f32, name="fsum", tag="fsum")
            nc.scalar.activation(e, t2, AF.Exp, bias=0.0, scale=1.0,
                                 accum_out=fsum)
            frec = small.tile([BC, 1], f32, name="frec", tag="frec")
            nc.vector.reciprocal(frec, fsum)
            wf = pfin.tile([BC, NK], f32, name="wf", tag="wf")
            nc.vector.tensor_scalar_mul(wf, e[:, 0:NK], frec)
            nc.gpsimd.dma_start(out=out_d[8 * G:8 * G + 4, :], in_=wf[0:4, :])
            nc.gpsimd.dma_start(out=out_d[8 * G + 4:8 * G + 8, :],
                                in_=wf[32:36, :])

    nc.compile()
    return nc


# ----------------------------------------------------------------------------
# host side
# ----------------------------------------------------------------------------

def _get_program(flags):
    if flags not in _cache:
        _cache[flags] = _build_program(flags)
    return _cache[flags]


def _prep_inputs(inputs):
    """Host-side sharding + layout prep; returns (flags, per-core input maps)."""
    f = np.float32
    h_t = np.asarray(inputs["h_t"], f)
    K_att = np.asarray(inputs["K_att"], f)
    V_att = np.asarray(inputs["V_att"], f)
    K_sa = np.asarray(inputs["K_sa"], f)
    V_sa = np.asarray(inputs["V_sa"], f)
    mask = np.asarray(inputs["mask"])

    sc = np.float32(DH ** -0.5)
    scf = np.float32(D ** -0.5)
    W = {n: np.asarray(inputs[n], f) for n in WNAMES}
    W["Wq_sa"] = W["Wq_sa"] * sc
    W["Wq_a"] = W["Wq_a"] * sc
    W["Wqf"] = W["Wqf"] * scf
    bias_src = {"Wq_sa": "bq_sa", "Wk_sa": "bk_sa", "Wv_sa": "bv_sa",
                "W0_sa": "b0_sa", "Wq_a": "bq_a", "W0_a": "b0_a",
                "W1": "b1", "W2": "b2", "Wqf": "bqf"}
    bvec = {n: np.asarray(inputs[bias_src[n]], f).copy() for n in WNAMES}
    bvec["Wq_sa"] *= sc
    bvec["Wq_a"] *= sc
    bvec["Wqf"] *= scf
    use_bias = tuple(bool(np.any(bvec[n])) for n in WNAMES)
    ub = dict(zip(WNAMES, use_bias))

    lnp = np.stack([np.asarray(inputs[k], f) for k in
                    ["ln1_g", "ln1_b", "ln2_g", "ln2_b", "ln3_g", "ln3_b"]])
    ln_affine = tuple(
        bool(np.any(lnp[2 * i] != 1.0) or np.any(lnp[2 * i + 1] != 0.0))
        for i in range(3))
    flags = (use_bias, ln_affine)

    # d-major weight slabs [128, 4, 512] bf16
    Wb = {n: np.ascontiguousarray(
        W[n].reshape(4, P, D).transpose(1, 0, 2)).astype(BF) for n in WNAMES}

    # streams, host-packed per pair of 4-sample groups (final: 8), bf16
    # ksa[core][G2, p, u, j, c, t] = K_sa[64c+8G2+4u+j, t, 128c+p]
    ksa = np.ascontiguousarray(
        K_sa.transpose(0, 2, 1).reshape(NCORES, NG2, 2, 4, 4, P, T)
        .transpose(0, 1, 5, 2, 3, 4, 6)).astype(BF)
    vsa = np.ascontiguousarray(
        V_sa.reshape(NCORES, NG2, 2, 4, 2, P, D)
        .transpose(0, 1, 5, 2, 3, 4, 6)).astype(BF)
    ka0t = np.zeros((B, D, NP), f)
    ka0t[:, :, :NK] = K_att[:, :, :D].transpose(0, 2, 1)
    ka0 = np.ascontiguousarray(
        ka0t.reshape(NCORES, NG2, 2, 4, 4, P, NP)
        .transpose(0, 1, 5, 2, 3, 4, 6)).astype(BF)
    va0p = np.zeros((B, NP, D), f)
    va0p[:, :NK, :] = V_att[:, :, :D]
    va0 = np.ascontiguousarray(
        va0p.reshape(NCORES, NG2, 2, 4, 2, P, D)
        .transpose(0, 1, 5, 2, 3, 4, 6)).astype(ml_dtypes.float8_e4m3)
    kaft = np.zeros((B, D, NP), f)
    kaft[:, :, :NK] = K_att[:, :, D:].transpose(0, 2, 1)
    kaf = np.ascontiguousarray(
        kaft.reshape(NCORES, NGF, 8, 4, P, NP)
        .transpose(0, 1, 4, 2, 3, 5)).astype(BF)

    maskadd = np.full((B, NP), -1e9, f)
    maskadd[:, :NK] = np.where(mask, f(-1e9), f(0.0))
    # mask2[core][g, p, n] = maskadd[64c + 4g + p//32, n]
    mask2 = np.ascontiguousarray(
        np.broadcast_to(maskadd.reshape(NCORES, NG, 4, 1, NP),
                        (NCORES, NG, 4, 32, NP)).reshape(NCORES, NG, P, NP))
    # maskF[core][G, p, n] = maskadd[64c + 8G + 4*(p//32) + min(p%32,3), n]
    p_arr = np.arange(BC)
    samp_idx = (8 * np.arange(NGF)[:, None] + 4 * (p_arr // 32)[None, :]
                + np.minimum(p_arr % 32, 3)[None, :])        # [NGF, 64]
    mc_ = maskadd.reshape(NCORES, BC, NP)
    maskF = np.ascontiguousarray(mc_[:, samp_idx, :])        # [core,NGF,64,NP]

    # constants
    identF = np.eye(P, dtype=f)
    identB = np.eye(P).astype(BF)
    obig = np.zeros((P, 124), f)
    for j in range(4):
        obig[32 * j:32 * j + H, 60 + j] = 1.0
    obig = obig.astype(BF)
    bmask4 = np.zeros((P, 4), f)
    for j in range(4):
        bmask4[32 * j:32 * j + 32, j] = 1.0
    bmask4 = bmask4.astype(BF)
    bm4 = np.zeros((P, D), f)
    for j in range(4):
        for hh in range(H):
            bm4[32 * j + hh, DH * hh:DH * (hh + 1)] = 1.0
    # qm[p, c, m] = 1 iff m == head(128c+p)
    qm = np.zeros((P, 4, DH), f)
    for c in range(4):
        for p in range(P):
            qm[p, c, (c * P + p) // DH] = 1.0
    qm = qm.astype(BF)
    pm4 = np.zeros((P, 8, 4), f)
    for s in range(8):
        pm4[:, s, s % 4] = 1.0
    pm4 = pm4.astype(BF)

    hT = np.ascontiguousarray(
        h_t.reshape(NCORES, BC, 4, P).transpose(0, 3, 2, 1)).astype(BF)

    b_dmaj = {n: np.ascontiguousarray(bvec[n].reshape(4, P).T) for n in WNAMES}

    in_maps = []
    for i in range(NCORES):
        sl = slice(BC * i, BC * (i + 1))
        m = {
            "hT": hT[i],
            "h_bm": np.ascontiguousarray(h_t[sl]),
            "ksa": ksa[i],
            "vsa": vsa[i],
            "ka0": ka0[i],
            "va0": va0[i],
            "kaf": kaf[i],
            "mask2": mask2[i],
            "maskF": maskF[i],
            "identF": identF,
            "identB": identB,
            "obig": obig,
            "bmask4": bmask4,
            "bm4": bm4,
            "qm": qm,
            "pm4": pm4,
        }
        for n in WNAMES:
            m["W_" + n] = Wb[n]
            if ub[n]:
                m["b_" + n] = b_dmaj[n]
                if n in BM_BIAS:
                    m["bf_" + n] = bvec[n].reshape(1, D)
        if any(ln_affine):
            m["lnp"] = lnp
        in_maps.append(m)
    return flags, in_maps


def _run(inputs, trace=False):
    flags, in_maps = _prep_inputs(inputs)
    nc = _get_program(flags)
    kwargs = {}
    if trace:
        kwargs = dict(trace=True, trace_cores=[0])
    res = run_bass_kernel_spmd(nc, in_maps, list(range(NCORES)), **kwargs)
    out = np.concatenate([res.results[i]["out"] for i in range(NCORES)], axis=0)
    return np.ascontiguousarray(out.astype(np.float32)), res


def kernel(**inputs):
    return _run(inputs, trace=False)[0]


def kernel_traced(**inputs):
    return _run(inputs, trace=True)


# revision 81
# speedup vs baseline: 1.1152x; 1.1152x over previous
"""Trainium2 Bass kernel for nn_AttentionModel (pointer-network decode step).

Data-parallel over 8 NeuronCores: batch 512 -> 64 samples/core; weights
replicated.  Per core the kernel streams the per-sample K/V slabs from HBM
once in bf16 (~10.6 MB/group-phase, ~90 MB total) and computes:

  self-attn over (K_sa | k_sa) -> LN -> enc attention (masked) -> LN ->
  MLP -> LN -> single-head tanh-clipped pointer scores -> softmax weights.

v2 (bf16 streaming) layout notes:
  - all K/V streams, weights and matmul activations are bf16; PSUM, LN and
    softmax normalization stay fp32.  Streams are host-packed per group of
    4 samples into contiguous [128, ...] slabs so each dma_start moves ~1MB.
  - the on-device-computed new-token key k_sa is copied into column 256 of
    the streamed K tile, so one matmul accumulation produces all 257 scores.
  - scores/softmax use no max subtraction (scores are bounded by ~±7 for
    this model: q rows are LN outputs times 0.05-scale weights).
  - per-group attention outputs are folded into a single [64, 512] PSUM
    accumulator via a shifting block-diagonal ones matrix (Obig), replacing
    the per-group [4,512] fold + SBUF + DMA round trip.
  - the final single-head layer packs 8 samples per PSUM tile using a
    4x4 one-hot placement mask (pm4) so each 32-row band holds 4 samples.
"""

import numpy as np
import ml_dtypes
from contextlib import ExitStack

import concourse.bass as bass
import concourse.tile as tile
from concourse import bacc, mybir
from concourse.bass_utils import run_bass_kernel_spmd

f32 = mybir.dt.float32
bf16 = mybir.dt.bfloat16
fp8 = mybir.dt.float8e4
AF = mybir.ActivationFunctionType
ALU = mybir.AluOpType
AX = mybir.AxisListType

BF = ml_dtypes.bfloat16

P = 128          # SBUF partitions
NCORES = 8
B = 512          # full batch
BC = B // NCORES # batch per core (64)
D = 512          # model dim
H = 16           # heads
DH = 32          # head dim
NK = 251         # encoder keys (nb_nodes + 1)
NP = 256         # encoder keys padded to 256
T = 256          # self-attn cache length (new token appended on device)
NG = BC // 4     # sample groups of 4 (one [128, n] psum tile each)
NG2 = BC // 8    # stream pair-groups (two groups per ~2MB dma)
NGF = BC // 8    # final-layer groups of 8 samples
WNAMES = ["Wq_sa", "Wk_sa", "Wv_sa", "W0_sa", "Wq_a", "W0_a", "W1", "W2", "Wqf"]
# weight matmuls whose bias is applied on batch-major [64, 512] rows
BM_BIAS = {"Wv_sa", "W0_sa", "W0_a", "W2"}

_cache = {}


# ----------------------------------------------------------------------------
# program builder
# ----------------------------------------------------------------------------

def _build_program(flags):
    """flags = (use_bias tuple aligned with WNAMES, ln_affine tuple of 3)."""
    use_bias = dict(zip(WNAMES, flags[0]))
    ln_affine = flags[1]

    nc = bacc.Bacc("TRN2", target_bir_lowering=False, debug=False)

    def din(name, shape, dt=f32):
        return nc.dram_tensor(name, shape, dt, kind="ExternalInput").ap()

    hT_d = din("hT", [P, 4, BC], bf16)
    hbm_d = din("h_bm", [BC, D])
    ksa_d = din("ksa", [NG2, P, 2, 4, 4, T], bf16)
    vsa_d = din("vsa", [NG2, P, 2, 4, 2, D], bf16)
    ka0_d = din("ka0", [NG2, P, 2, 4, 4, NP], bf16)
    va0_d = din("va0", [NG2, P, 2, 4, 2, D], fp8)
    kaf_d = din("kaf", [NGF, P, 8, 4, NP], bf16)
    mask2_d = din("mask2", [NG, P, NP])
    maskF_d = din("maskF", [NGF, BC, NP])
    W_d = {n: din("W_" + n, [P, 4, D], bf16) for n in WNAMES}
    b_d = {n: din("b_" + n, [P, 4]) for n in WNAMES if use_bias[n]}
    bf_d = {n: din("bf_" + n, [1, D]) for n in WNAMES
            if use_bias[n] and n in BM_BIAS}
    if any(ln_affine):
        lnp_d = din("lnp", [6, D])
    identF_d = din("identF", [P, P])
    identB_d = din("identB", [P, P], bf16)
    obig_d = din("obig", [P, 124], bf16)
    bmask4_d = din("bmask4", [P, 4], bf16)
    bm4_d = din("bm4", [P, D])
    qm_d = din("qm", [P, 4, DH], bf16)
    pm4_d = din("pm4", [P, 8, 4], bf16)

    out_d = nc.dram_tensor("out", [BC, NK], f32, kind="ExternalOutput").ap()

    def bcast_row(ap2d, i, n):
        row = ap2d[i:i + 1, :]
        return bass.AP(tensor=row.tensor, offset=row.offset,
                       ap=[[0, n]] + list(row.ap)[1:])

    with tile.TileContext(nc) as tc, ExitStack() as ctx:
        consts = ctx.enter_context(tc.tile_pool(name="consts", bufs=1))
        pwts = ctx.enter_context(tc.tile_pool(name="wts", bufs=4))
        acts = ctx.enter_context(tc.tile_pool(name="acts", bufs=1))
        small = ctx.enter_context(tc.tile_pool(name="small", bufs=8))
        big_tmp = ctx.enter_context(tc.tile_pool(name="big_tmp", bufs=1))
        # stream pools: co-resident so cross-phase DMA prefetch never blocks
        pk1 = ctx.enter_context(tc.tile_pool(name="l1k", bufs=3))
        pv1 = ctx.enter_context(tc.tile_pool(name="l1v", bufs=2))
        pk2 = ctx.enter_context(tc.tile_pool(name="l2k", bufs=2))
        pv2 = ctx.enter_context(tc.tile_pool(name="l2v", bufs=2))
        pkf = ctx.enter_context(tc.tile_pool(name="fk", bufs=3))
        # attention scratch
        pqbd = ctx.enter_context(tc.tile_pool(name="qbd", bufs=2))
        pesc = ctx.enter_context(tc.tile_pool(name="esc", bufs=2))
        pwt = ctx.enter_context(tc.tile_pool(name="wt", bufs=2))
        pex = ctx.enter_context(tc.tile_pool(name="ex", bufs=2))
        pvr = ctx.enter_context(tc.tile_pool(name="vr", bufs=2))
        pfin = ctx.enter_context(tc.tile_pool(name="fin", bufs=2))
        pmsk = ctx.enter_context(tc.tile_pool(name="msk", bufs=2))
        # PSUM pools -- every distinct tag costs bufs x 1 bank; 8 banks total:
        #   ps_sc(1) + ps_wt(2) + ps_pt(2) + ps_at(1) + pj(2) = 8
        psc = ctx.enter_context(tc.tile_pool(name="psc", bufs=2, space="PSUM"))
        pswt = ctx.enter_context(tc.tile_pool(name="pswt", bufs=2, space="PSUM"))
        pspt = ctx.enter_context(tc.tile_pool(name="pspt", bufs=2, space="PSUM"))
        ppj = ctx.enter_context(tc.tile_pool(name="ppj", bufs=2, space="PSUM"))
        pacc = ppj

        # ------------------------------------------------------------------
        # constants / weights (x0/h first: the initial projections need them)
        # ------------------------------------------------------------------
        x0T = acts.tile([P, 4, BC], bf16, name="x0T", tag="x0T")
        nc.sync.dma_start(out=x0T, in_=hT_d)
        h_bm = acts.tile([BC, D], f32, name="h_bm", tag="h_bm")
        nc.sync.dma_start(out=h_bm, in_=hbm_d)
        # touch every ACT function we use so tables load during startup
        eps = consts.tile([P, 1], f32, name="eps", tag="eps")
        nc.vector.memset(eps, 1e-5)
        warm = consts.tile([P, 1], f32, name="warm", tag="warm")
        for fn in (AF.Exp, AF.Tanh, AF.Sqrt, AF.Identity, AF.Relu):
            nc.scalar.activation(warm, eps, fn, bias=0.0, scale=1.0)

        # pre-issue the first self-attn stream tiles so they transfer while
        # the startup consts/projections run
        pre_kt, pre_vt = {}, {}
        pre_kt[0] = pk1.tile([P, 4, 4, 260], bf16, name="kt1", tag="kt1")
        nc.sync.dma_start(out=pre_kt[0][:, :, :, 0:T], in_=ksa_d[0][:, 0])
        pre_vt[0] = pv1.tile([P, 4, 2, D], bf16, name="vt1", tag="vt1")
        nc.sync.dma_start(out=pre_vt[0], in_=vsa_d[0][:, 0])

        identF = consts.tile([P, P], f32, name="identF", tag="identF")
        nc.sync.dma_start(out=identF, in_=identF_d)
        identB = consts.tile([P, P], bf16, name="identB", tag="identB")
        nc.sync.dma_start(out=identB, in_=identB_d)
        obig = consts.tile([P, 124], bf16, name="obig", tag="obig")
        nc.sync.dma_start(out=obig, in_=obig_d)
        bmask4 = consts.tile([P, 4], bf16, name="bmask4", tag="bmask4")
        nc.sync.dma_start(out=bmask4, in_=bmask4_d)
        bm4 = consts.tile([P, D], f32, name="bm4", tag="bm4")
        nc.sync.dma_start(out=bm4, in_=bm4_d)
        qm = consts.tile([P, 4, DH], bf16, name="qm", tag="qm")
        nc.sync.dma_start(out=qm, in_=qm_d)
        pm4 = consts.tile([P, 8, 4], bf16, name="pm4", tag="pm4")
        nc.sync.dma_start(out=pm4, in_=pm4_d)

        Wt, bt, bft = {}, {}, {}
        for n in WNAMES:
            Wt[n] = pwts.tile([P, 4, D], bf16, name="W_" + n, tag="W")
            nc.gpsimd.dma_start(out=Wt[n], in_=W_d[n])
            if use_bias[n]:
                bt[n] = consts.tile([P, 4], f32, name="b_" + n, tag="b_" + n)
                nc.sync.dma_start(out=bt[n], in_=b_d[n])
                if n in BM_BIAS:
                    bft[n] = consts.tile([BC, D], f32, name="bf_" + n, tag="bf_" + n)
                    nc.sync.dma_start(out=bft[n], in_=bcast_row(bf_d[n], 0, BC))

        lng, lnb = [None] * 3, [None] * 3
        for i in range(3):
            if ln_affine[i]:
                lng[i] = consts.tile([BC, D], f32, name=f"lng{i}", tag=f"lng{i}")
                nc.sync.dma_start(out=lng[i], in_=bcast_row(lnp_d, 2 * i, BC))
                lnb[i] = consts.tile([BC, D], f32, name=f"lnb{i}", tag=f"lnb{i}")
                nc.sync.dma_start(out=lnb[i], in_=bcast_row(lnp_d, 2 * i + 1, BC))

        # second l1 group + first enc-attn pair: prefetch behind the consts
        pre_kt[1] = pk1.tile([P, 4, 4, 260], bf16, name="kt1", tag="kt1")
        nc.sync.dma_start(out=pre_kt[1][:, :, :, 0:T], in_=ksa_d[0][:, 1])
        pre_vt[1] = pv1.tile([P, 4, 2, D], bf16, name="vt1", tag="vt1")
        nc.sync.dma_start(out=pre_vt[1], in_=vsa_d[0][:, 1])
        pre_kt2 = pk2.tile([P, 2, 4, 4, NP], bf16, name="kt2", tag="kt2")
        nc.sync.dma_start(out=pre_kt2, in_=ka0_d[0])
        pre_vt2 = pv2.tile([P, 2, 4, 2, D], fp8, name="vt2", tag="vt2")
        nc.sync.dma_start(out=pre_vt2, in_=va0_d[0])

        # ------------------------------------------------------------------
        # helpers
        # ------------------------------------------------------------------
        def proj_dmajor(dst, wname, src_T, relu=False):
            """dst[:, mc, :] (d-major [128, 4, 64] bf16) = act(x @ W + b)."""
            for mc in range(4):
                ps = ppj.tile([P, BC], f32, name="pp_" + wname, tag="pj")
                for kc in range(4):
                    nc.tensor.matmul(
                        ps,
                        lhsT=Wt[wname][:, kc, mc * P:(mc + 1) * P],
                        rhs=src_T[:, kc, :],
                        start=(kc == 0), stop=(kc == 3),
                    )
                bias = bt[wname][:, mc:mc + 1] if use_bias[wname] else 0.0
                func = AF.Relu if relu else AF.Identity
                nc.scalar.activation(dst[:, mc, :], ps, func, bias=bias, scale=1.0)

        def mm_batchmajor(ps, src_T, wname):
            """ps [64, 512] = x @ W   (lhsT = x^T chunks, W as moving)."""
            for kc in range(4):
                nc.tensor.matmul(
                    ps,
                    lhsT=src_T[:, kc, :],
                    rhs=Wt[wname][:, kc, :],
                    start=(kc == 0), stop=(kc == 3),
                )

        def transpose_bm(dst_T, src_bm):
            """[64, 512] f32 batch-major -> d-major [128, 4, 64] bf16."""
            for c in range(4):
                ps = pswt.tile([P, BC], f32, name="ptr", tag="ps_wt")
                nc.tensor.transpose(ps, src_bm[:, c * P:(c + 1) * P],
                                    identF[0:BC, 0:BC])
                nc.vector.tensor_copy(dst_T[:, c, :], ps)

        def layer_norm(dst_bm, t_bm, idx):
            stats = small.tile([BC, 6], f32, name="stats", tag="stats")
            nc.vector.bn_stats(stats, t_bm)
            mv = small.tile([BC, 2], f32, name="mv", tag="mv")
            nc.vector.bn_aggr(mv, stats)
            sd = small.tile([BC, 1], f32, name="sd", tag="sd")
            nc.scalar.activation(sd, mv[:, 1:2], AF.Sqrt, bias=eps[0:BC], scale=1.0)
            rstd = small.tile([BC, 1], f32, name="rstd", tag="rstd")
            nc.vector.reciprocal(rstd, sd)
            nmr = small.tile([BC, 1], f32, name="nmr", tag="nmr")
            nc.vector.scalar_tensor_tensor(out=nmr, in0=mv[:, 0:1], scalar=-1.0,
                                           in1=rstd, op0=ALU.mult, op1=ALU.mult)
            if ln_affine[idx]:
                xn = big_tmp.tile([BC, D], f32, name="xn", tag="xn")
                nc.scalar.activation(xn, t_bm, AF.Identity, bias=nmr, scale=rstd)
                nc.vector.tensor_mul(xn, xn, lng[idx])
                nc.vector.tensor_add(dst_bm, xn, lnb[idx])
            else:
                nc.scalar.activation(dst_bm, t_bm, AF.Identity, bias=nmr, scale=rstd)

        def residual_ln(dst_bm, dst_T, src_T, wname, x_prev_bm, idx):
            """dst = LN(x_prev + src @ W + b); also produce d-major dst_T."""
            ps = ppj.tile([BC, D], f32, name="pr_" + wname, tag="pj")
            mm_batchmajor(ps, src_T, wname)
            t_bm = big_tmp.tile([BC, D], f32, name="t_bm", tag="t_bm")
            nc.vector.tensor_add(t_bm, ps, x_prev_bm)
            if use_bias[wname]:
                nc.vector.tensor_add(t_bm, t_bm, bft[wname])
            layer_norm(dst_bm, t_bm, idx)
            transpose_bm(dst_T, dst_bm)

        def build_qbd(q_T, g):
            """[128, 4(j), 4(c), 32] bf16: sample 4g+j's q placed per-head."""
            qbd = pqbd.tile([P, 4, 4, DH], bf16, name="qbd", tag="qbd")
            in0 = q_T[:, :, 4 * g:4 * g + 4].transpose([0, 2, 1]) \
                .unsqueeze(3).broadcast_to([P, 4, 4, DH])
            in1 = qm.unsqueeze(1).broadcast_to([P, 4, 4, DH])
            nc.vector.tensor_mul(qbd, in0, in1)
            return qbd

        # ------------------------------------------------------------------
        # projections from x0 = h_t
        # ------------------------------------------------------------------
        q_saT = acts.tile([P, 4, BC], bf16, name="q_saT", tag="q_saT")
        proj_dmajor(q_saT, "Wq_sa", x0T)
        k_saT = acts.tile([P, 4, BC], bf16, name="k_saT", tag="k_saT")
        proj_dmajor(k_saT, "Wk_sa", x0T)

        v_bmb = acts.tile([BC, D], bf16, name="v_bmb", tag="v_bmb")
        psv = ppj.tile([BC, D], f32, name="psv", tag="pj")
        mm_batchmajor(psv, x0T, "Wv_sa")
        if use_bias["Wv_sa"]:
            tv = big_tmp.tile([BC, D], f32, name="tv", tag="tv")
            nc.vector.tensor_add(tv, psv, bft["Wv_sa"])
            nc.vector.tensor_copy(v_bmb, tv)
        else:
            nc.scalar.copy(v_bmb, psv)

        # ------------------------------------------------------------------
        # layer 1: self-attention over (K_sa | k_sa)
        # ------------------------------------------------------------------
        wcols = T + 1
        ps_at1 = pacc.tile([BC, D], f32, name="ps_at1", tag="pj")
        for g in range(NG):
            if g in pre_kt:
                kt, vt = pre_kt[g], pre_vt[g]
            else:
                # 260-col rows keep each (j, c) row 8-byte aligned in SBUF
                kt = pk1.tile([P, 4, 4, 260], bf16, name="kt1", tag="kt1")
                nc.sync.dma_start(out=kt[:, :, :, 0:T],
                                  in_=ksa_d[g // 2][:, g % 2])
                vt = pv1.tile([P, 4, 2, D], bf16, name="vt1", tag="vt1")
                nc.sync.dma_start(out=vt, in_=vsa_d[g // 2][:, g % 2])
            # new-token key column (k_saT is [P, 4(c), BC])
            nc.vector.tensor_copy(
                kt[:, :, :, T:T + 1],
                k_saT[:, :, 4 * g:4 * g + 4].transpose([0, 2, 1]).unsqueeze(3))
            qbd = build_qbd(q_saT, g)
            vr4 = pvr.tile([4, D], bf16, name="vr4", tag="vr")
            nc.gpsimd.dma_start(out=vr4, in_=v_bmb[4 * g:4 * g + 4, :])

            ps_sc = psc.tile([P, wcols], f32, name="ps_sc", tag="ps_sc")
            for j in range(4):
                for c in range(4):
                    nc.tensor.matmul(
                        ps_sc[32 * j:32 * j + 32, :],
                        lhsT=qbd[:, j, c, :],
                        rhs=kt[:, j, c, 0:wcols],
                        start=(c == 0), stop=(c == 3),
                        tile_position=(0, 32 * j))
            esc = pesc.tile([P, wcols], bf16, name="esc", tag="esc")
            sumexp = small.tile([P, 1], f32, name="sumexp", tag="sumexp")
            nc.scalar.activation(esc, ps_sc, AF.Exp, bias=0.0, scale=1.0,
                                 accum_out=sumexp)
            recip = small.tile([P, 1], f32, name="recip", tag="recip")
            nc.vector.reciprocal(recip, sumexp)
            # new-token softmax weights, banded: escb4[p, r] =
            # esc[p, 256] * (p//32 == r); transposed it becomes the K=4
            # block-diagonal lhsT for the new-token value product.
            escb4 = pesc.tile([P, 4], bf16, name="escb4", tag="escb4")
            nc.vector.tensor_mul(escb4, esc[:, T:T + 1].broadcast_to([P, 4]),
                                 bmask4)

            ps_wt = pswt.tile([P, 3, P], bf16, name="ps_wt", tag="ps_wt")
            wt = pwt.tile([P, 3, P], bf16, name="wt", tag="wt")
            for c, cw, src in [(0, P, esc[:, 0:P]), (1, P, esc[:, P:2 * P]),
                               (2, 4, escb4)]:
                nc.tensor.transpose(ps_wt[0:cw, c, :], src, identB)
                nc.vector.tensor_copy(wt[0:cw, c, :], ps_wt[0:cw, c, :])

            ps_pt = pspt.tile([P, D], f32, name="ps_pt", tag="ps_pt")
            for j in range(4):
                for kc in range(2):
                    nc.tensor.matmul(
                        ps_pt[32 * j:32 * j + 32, :],
                        lhsT=wt[:, kc, 32 * j:32 * j + 32],
                        rhs=vt[:, j, kc, :],
                        start=(kc == 0), stop=False,
                        tile_position=(0, 32 * j), skip_group_check=True)
            # new-token contribution last, so the band matmuls above never
            # wait on the (small, late) vr4 staging dma
            nc.tensor.matmul(ps_pt, lhsT=wt[0:4, 2, :], rhs=vr4,
                             start=False, stop=True, skip_group_check=True)
            ex = pex.tile([P, D], bf16, name="ex", tag="ex")
            nc.vector.scalar_tensor_tensor(
                out=ex, in0=ps_pt, scalar=recip, in1=bm4,
                op0=ALU.mult, op1=ALU.mult)
            nc.tensor.matmul(ps_at1, lhsT=obig[:, 60 - 4 * g:124 - 4 * g],
                             rhs=ex, start=(g == 0), stop=(g == NG - 1))

        attn1 = acts.tile([BC, D], f32, name="attn1", tag="attn1")
        nc.scalar.copy(attn1, ps_at1)
        x1_bm = acts.tile([BC, D], f32, name="x1_bm", tag="x1_bm")
        x1T = acts.tile([P, 4, BC], bf16, name="x1T", tag="x1T")
        attn1T = acts.tile([P, 4, BC], bf16, name="attn1T", tag="attn1T")
        transpose_bm(attn1T, attn1)
        residual_ln(x1_bm, x1T, attn1T, "W0_sa", h_bm, 0)

        # ------------------------------------------------------------------
        # layer 2: encoder attention (masked, padded keys)
        # ------------------------------------------------------------------
        q_aT = acts.tile([P, 4, BC], bf16, name="q_aT", tag="q_aT")
        proj_dmajor(q_aT, "Wq_a", x1T)

        ps_at2 = pacc.tile([BC, D], f32, name="ps_at2", tag="pj")
        for G2 in range(NG2):
            if G2 == 0:
                kt, vt = pre_kt2, pre_vt2
            else:
                kt = pk2.tile([P, 2, 4, 4, NP], bf16, name="kt2", tag="kt2")
                nc.sync.dma_start(out=kt, in_=ka0_d[G2])
                vt = pv2.tile([P, 2, 4, 2, D], fp8, name="vt2", tag="vt2")
                nc.sync.dma_start(out=vt, in_=va0_d[G2])
            for u in range(2):
                g = 2 * G2 + u
                qbd = build_qbd(q_aT, g)

                mt = pmsk.tile([P, NP], f32, name="mt", tag="m2")
                nc.sync.dma_start(out=mt, in_=mask2_d[g])

                ps_sc = psc.tile([P, NP], f32, name="ps_sc2", tag="ps_sc")
                for j in range(4):
                    for c in range(4):
                        nc.tensor.matmul(
                            ps_sc[32 * j:32 * j + 32, :],
                            lhsT=qbd[:, j, c, :],
                            rhs=kt[:, u, j, c, :],
                            start=(c == 0), stop=(c == 3),
                            tile_position=(0, 32 * j))
                nc.vector.tensor_add(ps_sc, ps_sc, mt)
                esc = pesc.tile([P, NP], bf16, name="esc2", tag="esc")
                sumexp = small.tile([P, 1], f32, name="sumexp2", tag="sumexp")
                nc.scalar.activation(esc, ps_sc, AF.Exp, bias=0.0, scale=1.0,
                                     accum_out=sumexp)
                recip = small.tile([P, 1], f32, name="recip2", tag="recip")
                nc.vector.reciprocal(recip, sumexp)

                ps_wt = pswt.tile([P, 2, P], bf16, name="ps_wt2", tag="ps_wt")
                wt = pwt.tile([P, 2, P], bf16, name="wt2", tag="wt")
                for c in range(2):
                    nc.tensor.transpose(ps_wt[:, c, :],
                                        esc[:, c * P:(c + 1) * P], identB)
                    nc.vector.tensor_copy(wt[:, c, :], ps_wt[:, c, :])

                ps_pt = pspt.tile([P, D], f32, name="ps_pt2", tag="ps_pt")
                for j in range(4):
                    for kc in range(2):
                        nc.tensor.matmul(
                            ps_pt[32 * j:32 * j + 32, :],
                            lhsT=wt[:, kc, 32 * j:32 * j + 32],
                            rhs=vt[:, u, j, kc, :],
                            start=(kc == 0), stop=(kc == 1),
                            tile_position=(0, 32 * j))
                ex = pex.tile([P, D], bf16, name="ex2", tag="ex")
                nc.vector.scalar_tensor_tensor(
                    out=ex, in0=ps_pt, scalar=recip, in1=bm4,
                    op0=ALU.mult, op1=ALU.mult)
                nc.tensor.matmul(ps_at2, lhsT=obig[:, 60 - 4 * g:124 - 4 * g],
                                 rhs=ex, start=(g == 0), stop=(g == NG - 1))

        attn2 = acts.tile([BC, D], f32, name="attn2", tag="attn2")
        nc.scalar.copy(attn2, ps_at2)
        x2_bm = acts.tile([BC, D], f32, name="x2_bm", tag="x2_bm")
        x2T = acts.tile([P, 4, BC], bf16, name="x2T", tag="x2T")
        attn2T = acts.tile([P, 4, BC], bf16, name="attn2T", tag="attn2T")
        transpose_bm(attn2T, attn2)
        residual_ln(x2_bm, x2T, attn2T, "W0_a", x1_bm, 1)

        # ------------------------------------------------------------------
        # MLP
        # ------------------------------------------------------------------
        h1T = acts.tile([P, 4, BC], bf16, name="h1T", tag="h1T")
        proj_dmajor(h1T, "W1", x2T, relu=True)
        x3_bm = acts.tile([BC, D], f32, name="x3_bm", tag="x3_bm")
        x3T = acts.tile([P, 4, BC], bf16, name="x3T", tag="x3T")
        residual_ln(x3_bm, x3T, h1T, "W2", x2_bm, 2)

        qfT = acts.tile([P, 4, BC], bf16, name="qfT", tag="qfT")
        proj_dmajor(qfT, "Wqf", x3T)

        # ------------------------------------------------------------------
        # final pointer scores: w = softmax(10*tanh(qf.K/sqrt(D)) + mask)
        # 8 samples per group: rows 32*q4 + r  (q4 in 0..2, r in 0..4)
        # ------------------------------------------------------------------
        for G in range(NGF):
            kf = pkf.tile([P, 8, 4, NP], bf16, name="kf", tag="kf")
            nc.gpsimd.dma_start(out=kf, in_=kaf_d[G])
            # qfb[p, c, s, r] = qfT[p, c, 8G+s] * (s%4 == r)
            qfb = pqbd.tile([P, 4, 8, 4], bf16, name="qfb", tag="qfb")
            in0 = qfT[:, :, 8 * G:8 * G + 8].unsqueeze(3) \
                .broadcast_to([P, 4, 8, 4])
            in1 = pm4.unsqueeze(1).broadcast_to([P, 4, 8, 4])
            nc.vector.tensor_mul(qfb, in0, in1)

            ps_f = psc.tile([BC, NP], f32, name="ps_f", tag="ps_sc")
            nc.vector.memset(ps_f, 0.0)
            for q4 in range(2):
                for r in range(4):
                    for c in range(4):
                        nc.tensor.matmul(
                            ps_f[32 * q4:32 * q4 + 4, :],
                            lhsT=qfb[:, c, 4 * q4 + r, :],
                            rhs=kf[:, 4 * q4 + r, c, :],
                            start=(r == 0 and c == 0), stop=(r == 3 and c == 3),
                            tile_position=(0, 32 * q4))
            mf = pmsk.tile([BC, NP], f32, name="mf", tag="mf")
            nc.sync.dma_start(out=mf, in_=maskF_d[G])
            t1 = pfin.tile([BC, NP], f32, name="t1", tag="t1")
            nc.scalar.activation(t1, ps_f, AF.Tanh, bias=0.0, scale=1.0)
            t2 = pfin.tile([BC, NP], f32, name="t2", tag="t2")
            nc.vector.scalar_tensor_tensor(out=t2, in0=t1, scalar=10.0,
                                           in1=mf,
                                           op0=ALU.mult, op1=ALU.add)
            e = pfin.tile([BC, NP], f32, name="e", tag="e")
            fsum = small.tile([BC, 1], f32, name="fsum", tag="fsum")
            nc.scalar.activation(e, t2, AF.Exp, bias=0.0, scale=1.0,
                                 accum_out=fsum)
            frec = small.tile([BC, 1], f32, name="frec", tag="frec")
            nc.vector.reciprocal(frec, fsum)
            wf = pfin.tile([BC, NK], f32, name="wf", tag="wf")
            nc.vector.tensor_scalar_mul(wf, e[:, 0:NK], frec)
            nc.gpsimd.dma_start(out=out_d[8 * G:8 * G + 4, :], in_=wf[0:4, :])
            nc.gpsimd.dma_start(out=out_d[8 * G + 4:8 * G + 8, :],
                                in_=wf[32:36, :])

    nc.compile()
    return nc


# ----------------------------------------------------------------------------
# host side
# ----------------------------------------------------------------------------

def _get_program(flags):
    if flags not in _cache:
        _cache[flags] = _build_program(flags)
    return _cache[flags]


def _prep_inputs(inputs):
    """Host-side sharding + layout prep; returns (flags, per-core input maps)."""
    f = np.float32
    h_t = np.asarray(inputs["h_t"], f)
    K_att = np.asarray(inputs["K_att"], f)
    V_att = np.asarray(inputs["V_att"], f)
    K_sa = np.asarray(inputs["K_sa"], f)
    V_sa = np.asarray(inputs["V_sa"], f)
    mask = np.asarray(inputs["mask"])

    sc = np.float32(DH ** -0.5)
    scf = np.float32(D ** -0.5)
    W = {n: np.asarray(inputs[n], f) for n in WNAMES}
    W["Wq_sa"] = W["Wq_sa"] * sc
    W["Wq_a"] = W["Wq_a"] * sc
    W["Wqf"] = W["Wqf"] * scf
    bias_src = {"Wq_sa": "bq_sa", "Wk_sa": "bk_sa", "Wv_sa": "bv_sa",
                "W0_sa": "b0_sa", "Wq_a": "bq_a", "W0_a": "b0_a",
                "W1": "b1", "W2": "b2", "Wqf": "bqf"}
    bvec = {n: np.asarray(inputs[bias_src[n]], f).copy() for n in WNAMES}
    bvec["Wq_sa"] *= sc
    bvec["Wq_a"] *= sc
    bvec["Wqf"] *= scf
    use_bias = tuple(bool(np.any(bvec[n])) for n in WNAMES)
    ub = dict(zip(WNAMES, use_bias))

    lnp = np.stack([np.asarray(inputs[k], f) for k in
                    ["ln1_g", "ln1_b", "ln2_g", "ln2_b", "ln3_g", "ln3_b"]])
    ln_affine = tuple(
        bool(np.any(lnp[2 * i] != 1.0) or np.any(lnp[2 * i + 1] != 0.0))
        for i in range(3))
    flags = (use_bias, ln_affine)

    # d-major weight slabs [128, 4, 512] bf16
    Wb = {n: np.ascontiguousarray(
        W[n].reshape(4, P, D).transpose(1, 0, 2)).astype(BF) for n in WNAMES}

    # streams, host-packed per pair of 4-sample groups (final: 8), bf16
    # ksa[core][G2, p, u, j, c, t] = K_sa[64c+8G2+4u+j, t, 128c+p]
    ksa = np.ascontiguousarray(
        K_sa.transpose(0, 2, 1).reshape(NCORES, NG2, 2, 4, 4, P, T)
        .transpose(0, 1, 5, 2, 3, 4, 6)).astype(BF)
    vsa = np.ascontiguousarray(
        V_sa.reshape(NCORES, NG2, 2, 4, 2, P, D)
        .transpose(0, 1, 5, 2, 3, 4, 6)).astype(BF)
    ka0t = np.zeros((B, D, NP), f)
    ka0t[:, :, :NK] = K_att[:, :, :D].transpose(0, 2, 1)
    ka0 = np.ascontiguousarray(
        ka0t.reshape(NCORES, NG2, 2, 4, 4, P, NP)
        .transpose(0, 1, 5, 2, 3, 4, 6)).astype(BF)
    va0p = np.zeros((B, NP, D), f)
    va0p[:, :NK, :] = V_att[:, :, :D]
    va0 = np.ascontiguousarray(
        va0p.reshape(NCORES, NG2, 2, 4, 2, P, D)
        .transpose(0, 1, 5, 2, 3, 4, 6)).astype(ml_dtypes.float8_e4m3)
    kaft = np.zeros((B, D, NP), f)
    kaft[:, :, :NK] = K_att[:, :, D:].transpose(0, 2, 1)
    kaf = np.ascontiguousarray(
        kaft.reshape(NCORES, NGF, 8, 4, P, NP)
        .transpose(0, 1, 4, 2, 3, 5)).astype(BF)

    maskadd = np.full((B, NP), -1e9, f)
    maskadd[:, :NK] = np.where(mask, f(-1e9), f(0.0))
    # mask2[core][g, p, n] = maskadd[64c + 4g + p//32, n]
    mask2 = np.ascontiguousarray(
        np.broadcast_to(maskadd.reshape(NCORES, NG, 4, 1, NP),
                        (NCORES, NG, 4, 32, NP)).reshape(NCORES, NG, P, NP))
    # maskF[core][G, p, n] = maskadd[64c + 8G + 4*(p//32) + min(p%32,3), n]
    p_arr = np.arange(BC)
    samp_idx = (8 * np.arange(NGF)[:, None] + 4 * (p_arr // 32)[None, :]
                + np.minimum(p_arr % 32, 3)[None, :])        # [NGF, 64]
    mc_ = maskadd.reshape(NCORES, BC, NP)
    maskF = np.ascontiguousarray(mc_[:, samp_idx, :])        # [core,NGF,64,NP]

    # constants
    identF = np.eye(P, dtype=f)
    identB = np.eye(P).astype(BF)
    obig = np.zeros((P, 124), f)
    for j in range(4):
        obig[32 * j:32 * j + H, 60 + j] = 1.0
    obig = obig.astype(BF)
    bmask4 = np.zeros((P, 4), f)
    for j in range(4):
        bmask4[32 * j:32 * j + 32, j] = 1.0
    bmask4 = bmask4.astype(BF)
    bm4 = np.zeros((P, D), f)
    for j in range(4):
        for hh in range(H):
            bm4[32 * j + hh, DH * hh:DH * (hh + 1)] = 1.0
    # qm[p, c, m] = 1 iff m == head(128c+p)
    qm = np.zeros((P, 4, DH), f)
    for c in range(4):
        for p in range(P):
            qm[p, c, (c * P + p) // DH] = 1.0
    qm = qm.astype(BF)
    pm4 = np.zeros((P, 8, 4), f)
    for s in range(8):
        pm4[:, s, s % 4] = 1.0
    pm4 = pm4.astype(BF)

    hT = np.ascontiguousarray(
        h_t.reshape(NCORES, BC, 4, P).transpose(0, 3, 2, 1)).astype(BF)

    b_dmaj = {n: np.ascontiguousarray(bvec[n].reshape(4, P).T) for n in WNAMES}

    in_maps = []
    for i in range(NCORES):
        sl = slice(BC * i, BC * (i + 1))
        m = {
            "hT": hT[i],
            "h_bm": np.ascontiguousarray(h_t[sl]),
            "ksa": ksa[i],
            "vsa": vsa[i],
            "ka0": ka0[i],
            "va0": va0[i],
            "kaf": kaf[i],
            "mask2": mask2[i],
            "maskF": maskF[i],
            "identF": identF,
            "identB": identB,
            "obig": obig,
            "bmask4": bmask4,
            "bm4": bm4,
            "qm": qm,
            "pm4": pm4,
        }
        for n in WNAMES:
            m["W_" + n] = Wb[n]
            if ub[n]:
                m["b_" + n] = b_dmaj[n]
                if n in BM_BIAS:
                    m["bf_" + n] = bvec[n].reshape(1, D)
        if any(ln_affine):
            m["lnp"] = lnp
        in_maps.append(m)
    return flags, in_maps


def _run(inputs, trace=False):
    flags, in_maps = _prep_inputs(inputs)
    nc = _get_program(flags)
    kwargs = {}
    if trace:
        kwargs = dict(trace=True, trace_cores=[0])
    res = run_bass_kernel_spmd(nc, in_maps, list(range(NCORES)), **kwargs)
    out = np.concatenate([res.results[i]["out"] for i in range(NCORES)], axis=0)
    return np.ascontiguousarray(out.astype(np.float32)), res


def kernel(**inputs):
    return _run(inputs, trace=False)[0]


def kernel_traced(**inputs):
    return _run(inputs, trace=True)


# revision 86
# speedup vs baseline: 1.1646x; 1.0443x over previous
"""Trainium2 Bass kernel for nn_AttentionModel (pointer-network decode step).

Data-parallel over 8 NeuronCores: batch 512 -> 64 samples/core; weights
replicated.  Per core the kernel streams the per-sample K/V slabs from HBM
once in bf16 (~10.6 MB/group-phase, ~90 MB total) and computes:

  self-attn over (K_sa | k_sa) -> LN -> enc attention (masked) -> LN ->
  MLP -> LN -> single-head tanh-clipped pointer scores -> softmax weights.

v2 (bf16 streaming) layout notes:
  - all K/V streams, weights and matmul activations are bf16; PSUM, LN and
    softmax normalization stay fp32.  Streams are host-packed per group of
    4 samples into contiguous [128, ...] slabs so each dma_start moves ~1MB.
  - the on-device-computed new-token key k_sa is copied into column 256 of
    the streamed K tile, so one matmul accumulation produces all 257 scores.
  - scores/softmax use no max subtraction (scores are bounded by ~±7 for
    this model: q rows are LN outputs times 0.05-scale weights).
  - per-group attention outputs are folded into a single [64, 512] PSUM
    accumulator via a shifting block-diagonal ones matrix (Obig), replacing
    the per-group [4,512] fold + SBUF + DMA round trip.
  - the final single-head layer packs 8 samples per PSUM tile using a
    4x4 one-hot placement mask (pm4) so each 32-row band holds 4 samples.
"""

import numpy as np
import ml_dtypes
from contextlib import ExitStack

import concourse.bass as bass
import concourse.tile as tile
from concourse import bacc, mybir
from concourse.bass_utils import run_bass_kernel_spmd

f32 = mybir.dt.float32
bf16 = mybir.dt.bfloat16
fp8 = mybir.dt.float8e4
AF = mybir.ActivationFunctionType
ALU = mybir.AluOpType
AX = mybir.AxisListType

BF = ml_dtypes.bfloat16

P = 128          # SBUF partitions
NCORES = 8
B = 512          # full batch
BC = B // NCORES # batch per core (64)
D = 512          # model dim
H = 16           # heads
DH = 32          # head dim
NK = 251         # encoder keys (nb_nodes + 1)
NP = 256         # encoder keys padded to 256
T = 256          # self-attn cache length (new token appended on device)
NG = BC // 4     # sample groups of 4 (one [128, n] psum tile each)
NG2 = BC // 8    # stream pair-groups (two groups per ~2MB dma)
NGF = BC // 8    # final-layer groups of 8 samples
WNAMES = ["Wq_sa", "Wk_sa", "Wv_sa", "W0_sa", "Wq_a", "W0_a", "W1", "W2", "Wqf"]
# weight matmuls whose bias is applied on batch-major [64, 512] rows
BM_BIAS = {"Wv_sa", "W0_sa", "W0_a", "W2"}

_cache = {}


# ----------------------------------------------------------------------------
# program builder
# ----------------------------------------------------------------------------

def _build_program(flags):
    """flags = (use_bias tuple aligned with WNAMES, ln_affine tuple of 3)."""
    use_bias = dict(zip(WNAMES, flags[0]))
    ln_affine = flags[1]

    nc = bacc.Bacc("TRN2", target_bir_lowering=False, debug=False)

    def din(name, shape, dt=f32):
        return nc.dram_tensor(name, shape, dt, kind="ExternalInput").ap()

    hT_d = din("hT", [P, 4, BC], bf16)
    hbm_d = din("h_bm", [BC, D])
    ksa_d = din("ksa", [NG2, P, 2, 4, 4, 260], bf16)
    vsa_d = din("vsa", [NG2, P, 2, 4, 2, D], bf16)
    ka0_d = din("ka0", [NG2, P, 2, 4, 4, NP], bf16)
    va0_d = din("va0", [NG2, P, 2, 4, 2, D], fp8)
    kaf_d = din("kaf", [NGF, P, 8, 4, NP], bf16)
    mask2_d = din("mask2", [NG, P, NP])
    maskF_d = din("maskF", [NGF, BC, NP])
    W_d = {n: din("W_" + n, [P, 4, D], bf16) for n in WNAMES}
    b_d = {n: din("b_" + n, [P, 4]) for n in WNAMES if use_bias[n]}
    bf_d = {n: din("bf_" + n, [1, D]) for n in WNAMES
            if use_bias[n] and n in BM_BIAS}
    if any(ln_affine):
        lnp_d = din("lnp", [6, D])
    identF_d = din("identF", [P, P])
    identB_d = din("identB", [P, P], bf16)
    obig_d = din("obig", [P, 124], bf16)
    bmask4_d = din("bmask4", [P, 4], bf16)
    bm4_d = din("bm4", [P, D])
    qm_d = din("qm", [P, 4, DH], bf16)
    pm4_d = din("pm4", [P, 8, 4], bf16)

    out_d = nc.dram_tensor("out", [BC, NK], f32, kind="ExternalOutput").ap()

    def bcast_row(ap2d, i, n):
        row = ap2d[i:i + 1, :]
        return bass.AP(tensor=row.tensor, offset=row.offset,
                       ap=[[0, n]] + list(row.ap)[1:])

    with tile.TileContext(nc) as tc, ExitStack() as ctx:
        consts = ctx.enter_context(tc.tile_pool(name="consts", bufs=1))
        pwts = ctx.enter_context(tc.tile_pool(name="wts", bufs=4))
        acts = ctx.enter_context(tc.tile_pool(name="acts", bufs=1))
        small = ctx.enter_context(tc.tile_pool(name="small", bufs=8))
        big_tmp = ctx.enter_context(tc.tile_pool(name="big_tmp", bufs=1))
        # stream pools: co-resident so cross-phase DMA prefetch never blocks
        pk1 = ctx.enter_context(tc.tile_pool(name="l1k", bufs=3))
        pv1 = ctx.enter_context(tc.tile_pool(name="l1v", bufs=2))
        pk2 = ctx.enter_context(tc.tile_pool(name="l2k", bufs=2))
        pv2 = ctx.enter_context(tc.tile_pool(name="l2v", bufs=2))
        pkf = ctx.enter_context(tc.tile_pool(name="fk", bufs=3))
        # attention scratch
        pqbd = ctx.enter_context(tc.tile_pool(name="qbd", bufs=2))
        pesc = ctx.enter_context(tc.tile_pool(name="esc", bufs=2))
        pwt = ctx.enter_context(tc.tile_pool(name="wt", bufs=2))
        pex = ctx.enter_context(tc.tile_pool(name="ex", bufs=2))
        pvr = ctx.enter_context(tc.tile_pool(name="vr", bufs=2))
        pfin = ctx.enter_context(tc.tile_pool(name="fin", bufs=2))
        pmsk = ctx.enter_context(tc.tile_pool(name="msk", bufs=2))
        # PSUM pools -- every distinct tag costs bufs x 1 bank; 8 banks total:
        #   ps_sc(1) + ps_wt(2) + ps_pt(2) + ps_at(1) + pj(2) = 8
        psc = ctx.enter_context(tc.tile_pool(name="psc", bufs=2, space="PSUM"))
        pswt = ctx.enter_context(tc.tile_pool(name="pswt", bufs=2, space="PSUM"))
        pspt = ctx.enter_context(tc.tile_pool(name="pspt", bufs=2, space="PSUM"))
        ppj = ctx.enter_context(tc.tile_pool(name="ppj", bufs=2, space="PSUM"))
        pacc = ppj

        # ------------------------------------------------------------------
        # constants / weights (x0/h first: the initial projections need them)
        # ------------------------------------------------------------------
        x0T = acts.tile([P, 4, BC], bf16, name="x0T", tag="x0T")
        nc.sync.dma_start(out=x0T, in_=hT_d)
        h_bm = acts.tile([BC, D], f32, name="h_bm", tag="h_bm")
        nc.sync.dma_start(out=h_bm, in_=hbm_d)
        # touch every ACT function we use so tables load during startup
        eps = consts.tile([P, 1], f32, name="eps", tag="eps")
        nc.vector.memset(eps, 1e-5)
        warm = consts.tile([P, 1], f32, name="warm", tag="warm")
        for fn in (AF.Exp, AF.Tanh, AF.Sqrt, AF.Identity, AF.Relu):
            nc.scalar.activation(warm, eps, fn, bias=0.0, scale=1.0)

        # pre-issue the first self-attn stream tiles so they transfer while
        # the startup consts/projections run
        pre_kt, pre_vt = {}, {}
        pre_kt[0] = pk1.tile([P, 4, 4, 260], bf16, name="kt1", tag="kt1")
        nc.sync.dma_start(out=pre_kt[0], in_=ksa_d[0][:, 0])
        pre_vt[0] = pv1.tile([P, 4, 2, D], bf16, name="vt1", tag="vt1")
        nc.sync.dma_start(out=pre_vt[0], in_=vsa_d[0][:, 0])

        identF = consts.tile([P, P], f32, name="identF", tag="identF")
        nc.sync.dma_start(out=identF, in_=identF_d)
        identB = consts.tile([P, P], bf16, name="identB", tag="identB")
        nc.sync.dma_start(out=identB, in_=identB_d)
        obig = consts.tile([P, 124], bf16, name="obig", tag="obig")
        nc.sync.dma_start(out=obig, in_=obig_d)
        bmask4 = consts.tile([P, 4], bf16, name="bmask4", tag="bmask4")
        nc.sync.dma_start(out=bmask4, in_=bmask4_d)
        bm4 = consts.tile([P, D], f32, name="bm4", tag="bm4")
        nc.sync.dma_start(out=bm4, in_=bm4_d)
        qm = consts.tile([P, 4, DH], bf16, name="qm", tag="qm")
        nc.sync.dma_start(out=qm, in_=qm_d)
        pm4 = consts.tile([P, 8, 4], bf16, name="pm4", tag="pm4")
        nc.sync.dma_start(out=pm4, in_=pm4_d)

        Wt, bt, bft = {}, {}, {}
        for n in WNAMES:
            Wt[n] = pwts.tile([P, 4, D], bf16, name="W_" + n, tag="W")
            nc.gpsimd.dma_start(out=Wt[n], in_=W_d[n])
            if use_bias[n]:
                bt[n] = consts.tile([P, 4], f32, name="b_" + n, tag="b_" + n)
                nc.sync.dma_start(out=bt[n], in_=b_d[n])
                if n in BM_BIAS:
                    bft[n] = consts.tile([BC, D], f32, name="bf_" + n, tag="bf_" + n)
                    nc.sync.dma_start(out=bft[n], in_=bcast_row(bf_d[n], 0, BC))

        lng, lnb = [None] * 3, [None] * 3
        for i in range(3):
            if ln_affine[i]:
                lng[i] = consts.tile([BC, D], f32, name=f"lng{i}", tag=f"lng{i}")
                nc.sync.dma_start(out=lng[i], in_=bcast_row(lnp_d, 2 * i, BC))
                lnb[i] = consts.tile([BC, D], f32, name=f"lnb{i}", tag=f"lnb{i}")
                nc.sync.dma_start(out=lnb[i], in_=bcast_row(lnp_d, 2 * i + 1, BC))

        # second l1 group + first enc-attn pair: prefetch behind the consts
        pre_kt[1] = pk1.tile([P, 4, 4, 260], bf16, name="kt1", tag="kt1")
        nc.sync.dma_start(out=pre_kt[1], in_=ksa_d[0][:, 1])
        pre_vt[1] = pv1.tile([P, 4, 2, D], bf16, name="vt1", tag="vt1")
        nc.sync.dma_start(out=pre_vt[1], in_=vsa_d[0][:, 1])
        pre_kt2 = pk2.tile([P, 2, 4, 4, NP], bf16, name="kt2", tag="kt2")
        nc.sync.dma_start(out=pre_kt2, in_=ka0_d[0])
        pre_vt2 = pv2.tile([P, 2, 4, 2, D], fp8, name="vt2", tag="vt2")
        nc.sync.dma_start(out=pre_vt2, in_=va0_d[0])

        # ------------------------------------------------------------------
        # helpers
        # ------------------------------------------------------------------
        def proj_dmajor(dst, wname, src_T, relu=False):
            """dst[:, mc, :] (d-major [128, 4, 64] bf16) = act(x @ W + b)."""
            for mc in range(4):
                ps = ppj.tile([P, BC], f32, name="pp_" + wname, tag="pj")
                for kc in range(4):
                    nc.tensor.matmul(
                        ps,
                        lhsT=Wt[wname][:, kc, mc * P:(mc + 1) * P],
                        rhs=src_T[:, kc, :],
                        start=(kc == 0), stop=(kc == 3),
                    )
                bias = bt[wname][:, mc:mc + 1] if use_bias[wname] else 0.0
                func = AF.Relu if relu else AF.Identity
                nc.scalar.activation(dst[:, mc, :], ps, func, bias=bias, scale=1.0)

        def mm_batchmajor(ps, src_T, wname):
            """ps [64, 512] = x @ W   (lhsT = x^T chunks, W as moving)."""
            for kc in range(4):
                nc.tensor.matmul(
                    ps,
                    lhsT=src_T[:, kc, :],
                    rhs=Wt[wname][:, kc, :],
                    start=(kc == 0), stop=(kc == 3),
                )

        def transpose_bm(dst_T, src_bm):
            """[64, 512] f32 batch-major -> d-major [128, 4, 64] bf16."""
            for c in range(4):
                ps = pswt.tile([P, BC], f32, name="ptr", tag="ps_wt")
                nc.tensor.transpose(ps, src_bm[:, c * P:(c + 1) * P],
                                    identF[0:BC, 0:BC])
                nc.vector.tensor_copy(dst_T[:, c, :], ps)

        def layer_norm(dst_bm, t_bm, idx):
            stats = small.tile([BC, 6], f32, name="stats", tag="stats")
            nc.vector.bn_stats(stats, t_bm)
            mv = small.tile([BC, 2], f32, name="mv", tag="mv")
            nc.vector.bn_aggr(mv, stats)
            sd = small.tile([BC, 1], f32, name="sd", tag="sd")
            nc.scalar.activation(sd, mv[:, 1:2], AF.Sqrt, bias=eps[0:BC], scale=1.0)
            rstd = small.tile([BC, 1], f32, name="rstd", tag="rstd")
            nc.vector.reciprocal(rstd, sd)
            nmr = small.tile([BC, 1], f32, name="nmr", tag="nmr")
            nc.vector.scalar_tensor_tensor(out=nmr, in0=mv[:, 0:1], scalar=-1.0,
                                           in1=rstd, op0=ALU.mult, op1=ALU.mult)
            if ln_affine[idx]:
                xn = big_tmp.tile([BC, D], f32, name="xn", tag="xn")
                nc.scalar.activation(xn, t_bm, AF.Identity, bias=nmr, scale=rstd)
                nc.vector.tensor_mul(xn, xn, lng[idx])
                nc.vector.tensor_add(dst_bm, xn, lnb[idx])
            else:
                nc.scalar.activation(dst_bm, t_bm, AF.Identity, bias=nmr, scale=rstd)

        def residual_ln(dst_bm, dst_T, src_T, wname, x_prev_bm, idx):
            """dst = LN(x_prev + src @ W + b); also produce d-major dst_T."""
            ps = ppj.tile([BC, D], f32, name="pr_" + wname, tag="pj")
            mm_batchmajor(ps, src_T, wname)
            t_bm = big_tmp.tile([BC, D], f32, name="t_bm", tag="t_bm")
            nc.vector.tensor_add(t_bm, ps, x_prev_bm)
            if use_bias[wname]:
                nc.vector.tensor_add(t_bm, t_bm, bft[wname])
            layer_norm(dst_bm, t_bm, idx)
            transpose_bm(dst_T, dst_bm)

        def build_qbd(q_T, g):
            """[128, 4(j), 4(c), 32] bf16: sample 4g+j's q placed per-head."""
            qbd = pqbd.tile([P, 4, 4, DH], bf16, name="qbd", tag="qbd")
            in0 = q_T[:, :, 4 * g:4 * g + 4].transpose([0, 2, 1]) \
                .unsqueeze(3).broadcast_to([P, 4, 4, DH])
            in1 = qm.unsqueeze(1).broadcast_to([P, 4, 4, DH])
            nc.vector.tensor_mul(qbd, in0, in1)
            return qbd

        # ------------------------------------------------------------------
        # projections from x0 = h_t
        # ------------------------------------------------------------------
        q_saT = acts.tile([P, 4, BC], bf16, name="q_saT", tag="q_saT")
        proj_dmajor(q_saT, "Wq_sa", x0T)
        k_saT = acts.tile([P, 4, BC], bf16, name="k_saT", tag="k_saT")
        proj_dmajor(k_saT, "Wk_sa", x0T)

        v_bmb = acts.tile([BC, D], bf16, name="v_bmb", tag="v_bmb")
        psv = ppj.tile([BC, D], f32, name="psv", tag="pj")
        mm_batchmajor(psv, x0T, "Wv_sa")
        if use_bias["Wv_sa"]:
            tv = big_tmp.tile([BC, D], f32, name="tv", tag="tv")
            nc.vector.tensor_add(tv, psv, bft["Wv_sa"])
            nc.vector.tensor_copy(v_bmb, tv)
        else:
            nc.scalar.copy(v_bmb, psv)

        # ------------------------------------------------------------------
        # layer 1: self-attention over (K_sa | k_sa)
        # ------------------------------------------------------------------
        wcols = T + 1
        ps_at1 = pacc.tile([BC, D], f32, name="ps_at1", tag="pj")
        for g in range(NG):
            if g in pre_kt:
                kt, vt = pre_kt[g], pre_vt[g]
            else:
                # 260-col rows (host zero-padded): contiguous dma, 8B-aligned
                kt = pk1.tile([P, 4, 4, 260], bf16, name="kt1", tag="kt1")
                nc.sync.dma_start(out=kt, in_=ksa_d[g // 2][:, g % 2])
                vt = pv1.tile([P, 4, 2, D], bf16, name="vt1", tag="vt1")
                nc.sync.dma_start(out=vt, in_=vsa_d[g // 2][:, g % 2])
            # new-token key column (k_saT is [P, 4(c), BC])
            nc.vector.tensor_copy(
                kt[:, :, :, T:T + 1],
                k_saT[:, :, 4 * g:4 * g + 4].transpose([0, 2, 1]).unsqueeze(3))
            qbd = build_qbd(q_saT, g)
            vr4 = pvr.tile([4, D], bf16, name="vr4", tag="vr")
            nc.gpsimd.dma_start(out=vr4, in_=v_bmb[4 * g:4 * g + 4, :])

            ps_sc = psc.tile([P, wcols], f32, name="ps_sc", tag="ps_sc")
            for j in range(4):
                for c in range(4):
                    nc.tensor.matmul(
                        ps_sc[32 * j:32 * j + 32, :],
                        lhsT=qbd[:, j, c, :],
                        rhs=kt[:, j, c, 0:wcols],
                        start=(c == 0), stop=(c == 3),
                        tile_position=(0, 32 * j))
            esc = pesc.tile([P, wcols], bf16, name="esc", tag="esc")
            sumexp = small.tile([P, 1], f32, name="sumexp", tag="sumexp")
            nc.scalar.activation(esc, ps_sc, AF.Exp, bias=0.0, scale=1.0,
                                 accum_out=sumexp)
            recip = small.tile([P, 1], f32, name="recip", tag="recip")
            nc.vector.reciprocal(recip, sumexp)
            # new-token softmax weights, banded: escb4[p, r] =
            # esc[p, 256] * (p//32 == r); transposed it becomes the K=4
            # block-diagonal lhsT for the new-token value product.
            escb4 = pesc.tile([P, 4], bf16, name="escb4", tag="escb4")
            nc.vector.tensor_mul(escb4, esc[:, T:T + 1].broadcast_to([P, 4]),
                                 bmask4)

            ps_wt = pswt.tile([P, 3, P], bf16, name="ps_wt", tag="ps_wt")
            wt = pwt.tile([P, 3, P], bf16, name="wt", tag="wt")
            for c, cw, src in [(0, P, esc[:, 0:P]), (1, P, esc[:, P:2 * P]),
                               (2, 4, escb4)]:
                nc.tensor.transpose(ps_wt[0:cw, c, :], src, identB)
                nc.vector.tensor_copy(wt[0:cw, c, :], ps_wt[0:cw, c, :])

            ps_pt = pspt.tile([P, D], f32, name="ps_pt", tag="ps_pt")
            for j in range(4):
                for kc in range(2):
                    nc.tensor.matmul(
                        ps_pt[32 * j:32 * j + 32, :],
                        lhsT=wt[:, kc, 32 * j:32 * j + 32],
                        rhs=vt[:, j, kc, :],
                        start=(kc == 0), stop=False,
                        tile_position=(0, 32 * j), skip_group_check=True)
            # new-token contribution last, so the band matmuls above never
            # wait on the (small, late) vr4 staging dma
            nc.tensor.matmul(ps_pt, lhsT=wt[0:4, 2, :], rhs=vr4,
                             start=False, stop=True, skip_group_check=True)
            ex = pex.tile([P, D], bf16, name="ex", tag="ex")
            nc.vector.scalar_tensor_tensor(
                out=ex, in0=ps_pt, scalar=recip, in1=bm4,
                op0=ALU.mult, op1=ALU.mult)
            nc.tensor.matmul(ps_at1, lhsT=obig[:, 60 - 4 * g:124 - 4 * g],
                             rhs=ex, start=(g == 0), stop=(g == NG - 1))

        attn1 = acts.tile([BC, D], f32, name="attn1", tag="attn1")
        nc.scalar.copy(attn1, ps_at1)
        x1_bm = acts.tile([BC, D], f32, name="x1_bm", tag="x1_bm")
        x1T = acts.tile([P, 4, BC], bf16, name="x1T", tag="x1T")
        attn1T = acts.tile([P, 4, BC], bf16, name="attn1T", tag="attn1T")
        transpose_bm(attn1T, attn1)
        residual_ln(x1_bm, x1T, attn1T, "W0_sa", h_bm, 0)

        # ------------------------------------------------------------------
        # layer 2: encoder attention (masked, padded keys)
        # ------------------------------------------------------------------
        q_aT = acts.tile([P, 4, BC], bf16, name="q_aT", tag="q_aT")
        proj_dmajor(q_aT, "Wq_a", x1T)

        ps_at2 = pacc.tile([BC, D], f32, name="ps_at2", tag="pj")
        for G2 in range(NG2):
            if G2 == 0:
                kt, vt = pre_kt2, pre_vt2
            else:
                kt = pk2.tile([P, 2, 4, 4, NP], bf16, name="kt2", tag="kt2")
                nc.sync.dma_start(out=kt, in_=ka0_d[G2])
                vt = pv2.tile([P, 2, 4, 2, D], fp8, name="vt2", tag="vt2")
                nc.sync.dma_start(out=vt, in_=va0_d[G2])
            for u in range(2):
                g = 2 * G2 + u
                qbd = build_qbd(q_aT, g)

                mt = pmsk.tile([P, NP], f32, name="mt", tag="m2")
                nc.sync.dma_start(out=mt, in_=mask2_d[g])

                ps_sc = psc.tile([P, NP], f32, name="ps_sc2", tag="ps_sc")
                for j in range(4):
                    for c in range(4):
                        nc.tensor.matmul(
                            ps_sc[32 * j:32 * j + 32, :],
                            lhsT=qbd[:, j, c, :],
                            rhs=kt[:, u, j, c, :],
                            start=(c == 0), stop=(c == 3),
                            tile_position=(0, 32 * j))
                nc.vector.tensor_add(ps_sc, ps_sc, mt)
                esc = pesc.tile([P, NP], bf16, name="esc2", tag="esc")
                sumexp = small.tile([P, 1], f32, name="sumexp2", tag="sumexp")
                nc.scalar.activation(esc, ps_sc, AF.Exp, bias=0.0, scale=1.0,
                                     accum_out=sumexp)
                recip = small.tile([P, 1], f32, name="recip2", tag="recip")
                nc.vector.reciprocal(recip, sumexp)

                ps_wt = pswt.tile([P, 2, P], bf16, name="ps_wt2", tag="ps_wt")
                wt = pwt.tile([P, 2, P], bf16, name="wt2", tag="wt")
                for c in range(2):
                    nc.tensor.transpose(ps_wt[:, c, :],
                                        esc[:, c * P:(c + 1) * P], identB)
                    nc.vector.tensor_copy(wt[:, c, :], ps_wt[:, c, :])

                ps_pt = pspt.tile([P, D], f32, name="ps_pt2", tag="ps_pt")
                for j in range(4):
                    for kc in range(2):
                        nc.tensor.matmul(
                            ps_pt[32 * j:32 * j + 32, :],
                            lhsT=wt[:, kc, 32 * j:32 * j + 32],
                            rhs=vt[:, u, j, kc, :],
                            start=(kc == 0), stop=(kc == 1),
                            tile_position=(0, 32 * j))
                ex = pex.tile([P, D], bf16, name="ex2", tag="ex")
                nc.vector.scalar_tensor_tensor(
                    out=ex, in0=ps_pt, scalar=recip, in1=bm4,
                    op0=ALU.mult, op1=ALU.mult)
                nc.tensor.matmul(ps_at2, lhsT=obig[:, 60 - 4 * g:124 - 4 * g],
                                 rhs=ex, start=(g == 0), stop=(g == NG - 1))

        attn2 = acts.tile([BC, D], f32, name="attn2", tag="attn2")
        nc.scalar.copy(attn2, ps_at2)
        x2_bm = acts.tile([BC, D], f32, name="x2_bm", tag="x2_bm")
        x2T = acts.tile([P, 4, BC], bf16, name="x2T", tag="x2T")
        attn2T = acts.tile([P, 4, BC], bf16, name="attn2T", tag="attn2T")
        transpose_bm(attn2T, attn2)
        residual_ln(x2_bm, x2T, attn2T, "W0_a", x1_bm, 1)

        # ------------------------------------------------------------------
        # MLP
        # ------------------------------------------------------------------
        h1T = acts.tile([P, 4, BC], bf16, name="h1T", tag="h1T")
        proj_dmajor(h1T, "W1", x2T, relu=True)
        x3_bm = acts.tile([BC, D], f32, name="x3_bm", tag="x3_bm")
        x3T = acts.tile([P, 4, BC], bf16, name="x3T", tag="x3T")
        residual_ln(x3_bm, x3T, h1T, "W2", x2_bm, 2)

        qfT = acts.tile([P, 4, BC], bf16, name="qfT", tag="qfT")
        proj_dmajor(qfT, "Wqf", x3T)

        # ------------------------------------------------------------------
        # final pointer scores: w = softmax(10*tanh(qf.K/sqrt(D)) + mask)
        # 8 samples per group: rows 32*q4 + r  (q4 in 0..2, r in 0..4)
        # ------------------------------------------------------------------
        for G in range(NGF):
            kf = pkf.tile([P, 8, 4, NP], bf16, name="kf", tag="kf")
            nc.gpsimd.dma_start(out=kf, in_=kaf_d[G])
            # qfb[p, c, s, r] = qfT[p, c, 8G+s] * (s%4 == r)
            qfb = pqbd.tile([P, 4, 8, 4], bf16, name="qfb", tag="qfb")
            in0 = qfT[:, :, 8 * G:8 * G + 8].unsqueeze(3) \
                .broadcast_to([P, 4, 8, 4])
            in1 = pm4.unsqueeze(1).broadcast_to([P, 4, 8, 4])
            nc.vector.tensor_mul(qfb, in0, in1)

            ps_f = psc.tile([BC, NP], f32, name="ps_f", tag="ps_sc")
            nc.vector.memset(ps_f, 0.0)
            for q4 in range(2):
                for r in range(4):
                    for c in range(4):
                        nc.tensor.matmul(
                            ps_f[32 * q4:32 * q4 + 4, :],
                            lhsT=qfb[:, c, 4 * q4 + r, :],
                            rhs=kf[:, 4 * q4 + r, c, :],
                            start=(r == 0 and c == 0), stop=(r == 3 and c == 3),
                            tile_position=(0, 32 * q4))
            mf = pmsk.tile([BC, NP], f32, name="mf", tag="mf")
            nc.sync.dma_start(out=mf, in_=maskF_d[G])
            t1 = pfin.tile([BC, NP], f32, name="t1", tag="t1")
            nc.scalar.activation(t1, ps_f, AF.Tanh, bias=0.0, scale=1.0)
            t2 = pfin.tile([BC, NP], f32, name="t2", tag="t2")
            nc.vector.scalar_tensor_tensor(out=t2, in0=t1, scalar=10.0,
                                           in1=mf,
                                           op0=ALU.mult, op1=ALU.add)
            e = pfin.tile([BC, NP], f32, name="e", tag="e")
            fsum = small.tile([BC, 1], f32, name="fsum", tag="fsum")
            nc.scalar.activation(e, t2, AF.Exp, bias=0.0, scale=1.0,
                                 accum_out=fsum)
            frec = small.tile([BC, 1], f32, name="frec", tag="frec")
            nc.vector.reciprocal(frec, fsum)
            wf = pfin.tile([BC, NK], f32, name="wf", tag="wf")
            nc.vector.tensor_scalar_mul(wf, e[:, 0:NK], frec)
            nc.gpsimd.dma_start(out=out_d[8 * G:8 * G + 4, :], in_=wf[0:4, :])
            nc.gpsimd.dma_start(out=out_d[8 * G + 4:8 * G + 8, :],
                                in_=wf[32:36, :])

    nc.compile()
    return nc


# ----------------------------------------------------------------------------
# host side
# ----------------------------------------------------------------------------

def _get_program(flags):
    if flags not in _cache:
        _cache[flags] = _build_program(flags)
    return _cache[flags]


def _prep_inputs(inputs):
    """Host-side sharding + layout prep; returns (flags, per-core input maps)."""
    f = np.float32
    h_t = np.asarray(inputs["h_t"], f)
    K_att = np.asarray(inputs["K_att"], f)
    V_att = np.asarray(inputs["V_att"], f)
    K_sa = np.asarray(inputs["K_sa"], f)
    V_sa = np.asarray(inputs["V_sa"], f)
    mask = np.asarray(inputs["mask"])

    sc = np.float32(DH ** -0.5)
    scf = np.float32(D ** -0.5)
    W = {n: np.asarray(inputs[n], f) for n in WNAMES}
    W["Wq_sa"] = W["Wq_sa"] * sc
    W["Wq_a"] = W["Wq_a"] * sc
    W["Wqf"] = W["Wqf"] * scf
    bias_src = {"Wq_sa": "bq_sa", "Wk_sa": "bk_sa", "Wv_sa": "bv_sa",
                "W0_sa": "b0_sa", "Wq_a": "bq_a", "W0_a": "b0_a",
                "W1": "b1", "W2": "b2", "Wqf": "bqf"}
    bvec = {n: np.asarray(inputs[bias_src[n]], f).copy() for n in WNAMES}
    bvec["Wq_sa"] *= sc
    bvec["Wq_a"] *= sc
    bvec["Wqf"] *= scf
    use_bias = tuple(bool(np.any(bvec[n])) for n in WNAMES)
    ub = dict(zip(WNAMES, use_bias))

    lnp = np.stack([np.asarray(inputs[k], f) for k in
                    ["ln1_g", "ln1_b", "ln2_g", "ln2_b", "ln3_g", "ln3_b"]])
    ln_affine = tuple(
        bool(np.any(lnp[2 * i] != 1.0) or np.any(lnp[2 * i + 1] != 0.0))
        for i in range(3))
    flags = (use_bias, ln_affine)

    # d-major weight slabs [128, 4, 512] bf16
    Wb = {n: np.ascontiguousarray(
        W[n].reshape(4, P, D).transpose(1, 0, 2)).astype(BF) for n in WNAMES}

    # streams, host-packed per pair of 4-sample groups (final: 8), bf16
    # ksa[core][G2, p, u, j, c, t] = K_sa[64c+8G2+4u+j, t, 128c+p]; keys
    # padded 256 -> 260 so the kt dma is one contiguous run per partition
    ksa = np.zeros((NCORES, NG2, P, 2, 4, 4, 260), BF)
    ksa[..., :T] = (
        K_sa.transpose(0, 2, 1).reshape(NCORES, NG2, 2, 4, 4, P, T)
        .transpose(0, 1, 5, 2, 3, 4, 6)).astype(BF)
    vsa = np.ascontiguousarray(
        V_sa.reshape(NCORES, NG2, 2, 4, 2, P, D)
        .transpose(0, 1, 5, 2, 3, 4, 6)).astype(BF)
    ka0t = np.zeros((B, D, NP), f)
    ka0t[:, :, :NK] = K_att[:, :, :D].transpose(0, 2, 1)
    ka0 = np.ascontiguousarray(
        ka0t.reshape(NCORES, NG2, 2, 4, 4, P, NP)
        .transpose(0, 1, 5, 2, 3, 4, 6)).astype(BF)
    va0p = np.zeros((B, NP, D), f)
    va0p[:, :NK, :] = V_att[:, :, :D]
    va0 = np.ascontiguousarray(
        va0p.reshape(NCORES, NG2, 2, 4, 2, P, D)
        .transpose(0, 1, 5, 2, 3, 4, 6)).astype(ml_dtypes.float8_e4m3)
    kaft = np.zeros((B, D, NP), f)
    kaft[:, :, :NK] = K_att[:, :, D:].transpose(0, 2, 1)
    kaf = np.ascontiguousarray(
        kaft.reshape(NCORES, NGF, 8, 4, P, NP)
        .transpose(0, 1, 4, 2, 3, 5)).astype(BF)

    maskadd = np.full((B, NP), -1e9, f)
    maskadd[:, :NK] = np.where(mask, f(-1e9), f(0.0))
    # mask2[core][g, p, n] = maskadd[64c + 4g + p//32, n]
    mask2 = np.ascontiguousarray(
        np.broadcast_to(maskadd.reshape(NCORES, NG, 4, 1, NP),
                        (NCORES, NG, 4, 32, NP)).reshape(NCORES, NG, P, NP))
    # maskF[core][G, p, n] = maskadd[64c + 8G + 4*(p//32) + min(p%32,3), n]
    p_arr = np.arange(BC)
    samp_idx = (8 * np.arange(NGF)[:, None] + 4 * (p_arr // 32)[None, :]
                + np.minimum(p_arr % 32, 3)[None, :])        # [NGF, 64]
    mc_ = maskadd.reshape(NCORES, BC, NP)
    maskF = np.ascontiguousarray(mc_[:, samp_idx, :])        # [core,NGF,64,NP]

    # constants
    identF = np.eye(P, dtype=f)
    identB = np.eye(P).astype(BF)
    obig = np.zeros((P, 124), f)
    for j in range(4):
        obig[32 * j:32 * j + H, 60 + j] = 1.0
    obig = obig.astype(BF)
    bmask4 = np.zeros((P, 4), f)
    for j in range(4):
        bmask4[32 * j:32 * j + 32, j] = 1.0
    bmask4 = bmask4.astype(BF)
    bm4 = np.zeros((P, D), f)
    for j in range(4):
        for hh in range(H):
            bm4[32 * j + hh, DH * hh:DH * (hh + 1)] = 1.0
    # qm[p, c, m] = 1 iff m == head(128c+p)
    qm = np.zeros((P, 4, DH), f)
    for c in range(4):
        for p in range(P):
            qm[p, c, (c * P + p) // DH] = 1.0
    qm = qm.astype(BF)
    pm4 = np.zeros((P, 8, 4), f)
    for s in range(8):
        pm4[:, s, s % 4] = 1.0
    pm4 = pm4.astype(BF)

    hT = np.ascontiguousarray(
        h_t.reshape(NCORES, BC, 4, P).transpose(0, 3, 2, 1)).astype(BF)

    b_dmaj = {n: np.ascontiguousarray(bvec[n].reshape(4, P).T) for n in WNAMES}

    in_maps = []
    for i in range(NCORES):
        sl = slice(BC * i, BC * (i + 1))
        m = {
            "hT": hT[i],
            "h_bm": np.ascontiguousarray(h_t[sl]),
            "ksa": ksa[i],
            "vsa": vsa[i],
            "ka0": ka0[i],
            "va0": va0[i],
            "kaf": kaf[i],
            "mask2": mask2[i],
            "maskF": maskF[i],
            "identF": identF,
            "identB": identB,
            "obig": obig,
            "bmask4": bmask4,
            "bm4": bm4,
            "qm": qm,
            "pm4": pm4,
        }
        for n in WNAMES:
            m["W_" + n] = Wb[n]
            if ub[n]:
                m["b_" + n] = b_dmaj[n]
                if n in BM_BIAS:
                    m["bf_" + n] = bvec[n].reshape(1, D)
        if any(ln_affine):
            m["lnp"] = lnp
        in_maps.append(m)
    return flags, in_maps


def _run(inputs, trace=False):
    flags, in_maps = _prep_inputs(inputs)
    nc = _get_program(flags)
    kwargs = {}
    if trace:
        kwargs = dict(trace=True, trace_cores=[0])
    res = run_bass_kernel_spmd(nc, in_maps, list(range(NCORES)), **kwargs)
    out = np.concatenate([res.results[i]["out"] for i in range(NCORES)], axis=0)
    return np.ascontiguousarray(out.astype(np.float32)), res


def kernel(**inputs):
    return _run(inputs, trace=False)[0]


def kernel_traced(**inputs):
    return _run(inputs, trace=True)


# revision 90
# speedup vs baseline: 1.2368x; 1.0619x over previous
"""Trainium2 Bass kernel for nn_AttentionModel (pointer-network decode step).

Data-parallel over 8 NeuronCores: batch 512 -> 64 samples/core; weights
replicated.  Per core the kernel streams the per-sample K/V slabs from HBM
once in bf16 (~10.6 MB/group-phase, ~90 MB total) and computes:

  self-attn over (K_sa | k_sa) -> LN -> enc attention (masked) -> LN ->
  MLP -> LN -> single-head tanh-clipped pointer scores -> softmax weights.

v2 (bf16 streaming) layout notes:
  - all K/V streams, weights and matmul activations are bf16; PSUM, LN and
    softmax normalization stay fp32.  Streams are host-packed per group of
    4 samples into contiguous [128, ...] slabs so each dma_start moves ~1MB.
  - the on-device-computed new-token key k_sa is copied into column 256 of
    the streamed K tile, so one matmul accumulation produces all 257 scores.
  - scores/softmax use no max subtraction (scores are bounded by ~±7 for
    this model: q rows are LN outputs times 0.05-scale weights).
  - per-group attention outputs are folded into a single [64, 512] PSUM
    accumulator via a shifting block-diagonal ones matrix (Obig), replacing
    the per-group [4,512] fold + SBUF + DMA round trip.
  - the final single-head layer packs 8 samples per PSUM tile using a
    4x4 one-hot placement mask (pm4) so each 32-row band holds 4 samples.
"""

import numpy as np
import ml_dtypes
from contextlib import ExitStack

import concourse.bass as bass
import concourse.tile as tile
from concourse import bacc, mybir
from concourse.bass_utils import run_bass_kernel_spmd

f32 = mybir.dt.float32
bf16 = mybir.dt.bfloat16
fp8 = mybir.dt.float8e4
AF = mybir.ActivationFunctionType
ALU = mybir.AluOpType
AX = mybir.AxisListType

BF = ml_dtypes.bfloat16

P = 128          # SBUF partitions
NCORES = 8
B = 512          # full batch
BC = B // NCORES # batch per core (64)
D = 512          # model dim
H = 16           # heads
DH = 32          # head dim
NK = 251         # encoder keys (nb_nodes + 1)
NP = 256         # encoder keys padded to 256
T = 256          # self-attn cache length (new token appended on device)
NG = BC // 4     # sample groups of 4 (one [128, n] psum tile each)
NG2 = BC // 8    # stream pair-groups (two groups per ~2MB dma)
NGF = BC // 8    # final-layer groups of 8 samples
WNAMES = ["Wq_sa", "Wk_sa", "Wv_sa", "W0_sa", "Wq_a", "W0_a", "W1", "W2", "Wqf"]
# weight matmuls whose bias is applied on batch-major [64, 512] rows
BM_BIAS = {"Wv_sa", "W0_sa", "W0_a", "W2"}

_cache = {}


# ----------------------------------------------------------------------------
# program builder
# ----------------------------------------------------------------------------

def _build_program(flags):
    """flags = (use_bias tuple aligned with WNAMES, ln_affine tuple of 3)."""
    use_bias = dict(zip(WNAMES, flags[0]))
    ln_affine = flags[1]

    nc = bacc.Bacc("TRN2", target_bir_lowering=False, debug=False)

    def din(name, shape, dt=f32):
        return nc.dram_tensor(name, shape, dt, kind="ExternalInput").ap()

    hT_d = din("hT", [P, 4, BC], bf16)
    hbm_d = din("h_bm", [BC, D])
    ksa_d = din("ksa", [NG2, P, 2, 4, 4, 260], bf16)
    vsa_d = din("vsa", [NG2, P, 2, 4, 2, D], bf16)
    ka0_d = din("ka0", [NG2, P, 2, 4, 4, NP], bf16)
    va0_d = din("va0", [NG2, P, 2, 4, 2, D], fp8)
    kaf_d = din("kaf", [NGF, P, 8, 4, NP], bf16)
    mask2_d = din("mask2", [NG, P, NP])
    maskF_d = din("maskF", [NGF, BC, NP])
    W_d = {n: din("W_" + n, [P, 4, D], bf16) for n in WNAMES}
    b_d = {n: din("b_" + n, [P, 4]) for n in WNAMES if use_bias[n]}
    bf_d = {n: din("bf_" + n, [1, D]) for n in WNAMES
            if use_bias[n] and n in BM_BIAS}
    if any(ln_affine):
        lnp_d = din("lnp", [6, D])
    identF_d = din("identF", [P, P])
    identB_d = din("identB", [P, P], bf16)
    obig_d = din("obig", [P, 124], bf16)
    bmask4_d = din("bmask4", [P, 4], bf16)
    bm4_d = din("bm4", [P, D])
    qm_d = din("qm", [P, 4, DH], bf16)
    pm4_d = din("pm4", [P, 8, 4], bf16)

    out_d = nc.dram_tensor("out", [BC, NK], f32, kind="ExternalOutput").ap()

    def bcast_row(ap2d, i, n):
        row = ap2d[i:i + 1, :]
        return bass.AP(tensor=row.tensor, offset=row.offset,
                       ap=[[0, n]] + list(row.ap)[1:])

    with tile.TileContext(nc) as tc, ExitStack() as ctx:
        consts = ctx.enter_context(tc.tile_pool(name="consts", bufs=1))
        pwts = ctx.enter_context(tc.tile_pool(name="wts", bufs=4))
        acts = ctx.enter_context(tc.tile_pool(name="acts", bufs=1))
        small = ctx.enter_context(tc.tile_pool(name="small", bufs=8))
        big_tmp = ctx.enter_context(tc.tile_pool(name="big_tmp", bufs=1))
        # stream pools: co-resident so cross-phase DMA prefetch never blocks
        pk1 = ctx.enter_context(tc.tile_pool(name="l1k", bufs=3))
        pv1 = ctx.enter_context(tc.tile_pool(name="l1v", bufs=2))
        pk2 = ctx.enter_context(tc.tile_pool(name="l2k", bufs=2))
        pv2 = ctx.enter_context(tc.tile_pool(name="l2v", bufs=2))
        pkf = ctx.enter_context(tc.tile_pool(name="fk", bufs=3))
        # attention scratch
        pqbd = ctx.enter_context(tc.tile_pool(name="qbd", bufs=2))
        pesc = ctx.enter_context(tc.tile_pool(name="esc", bufs=2))
        pwt = ctx.enter_context(tc.tile_pool(name="wt", bufs=2))
        pex = ctx.enter_context(tc.tile_pool(name="ex", bufs=2))
        pvr = ctx.enter_context(tc.tile_pool(name="vr", bufs=2))
        pfin = ctx.enter_context(tc.tile_pool(name="fin", bufs=2))
        pmsk = ctx.enter_context(tc.tile_pool(name="msk", bufs=2))
        # PSUM pools -- every distinct tag costs bufs x 1 bank; 8 banks total:
        #   ps_sc(1) + ps_wt(2) + ps_pt(2) + ps_at(1) + pj(2) = 8
        psc = ctx.enter_context(tc.tile_pool(name="psc", bufs=2, space="PSUM"))
        pswt = ctx.enter_context(tc.tile_pool(name="pswt", bufs=2, space="PSUM"))
        pspt = ctx.enter_context(tc.tile_pool(name="pspt", bufs=2, space="PSUM"))
        ppj = ctx.enter_context(tc.tile_pool(name="ppj", bufs=2, space="PSUM"))
        pacc = ppj

        # ------------------------------------------------------------------
        # constants / weights (x0/h first: the initial projections need them)
        # ------------------------------------------------------------------
        x0T = acts.tile([P, 4, BC], bf16, name="x0T", tag="x0T")
        nc.sync.dma_start(out=x0T, in_=hT_d)
        h_bm = acts.tile([BC, D], f32, name="h_bm", tag="h_bm")
        nc.sync.dma_start(out=h_bm, in_=hbm_d)
        # touch every ACT function we use so tables load during startup
        eps = consts.tile([P, 1], f32, name="eps", tag="eps")
        nc.vector.memset(eps, 1e-5)
        warm = consts.tile([P, 1], f32, name="warm", tag="warm")
        for fn in (AF.Exp, AF.Tanh, AF.Sqrt, AF.Identity, AF.Relu):
            nc.scalar.activation(warm, eps, fn, bias=0.0, scale=1.0)

        # pre-issue the first self-attn stream tiles so they transfer while
        # the startup consts/projections run
        pre_kt, pre_vt = {}, {}
        pre_kt[0] = pk1.tile([P, 4, 4, 260], bf16, name="kt1", tag="kt1")
        nc.sync.dma_start(out=pre_kt[0], in_=ksa_d[0][:, 0])
        pre_vt[0] = pv1.tile([P, 4, 2, D], bf16, name="vt1", tag="vt1")
        nc.sync.dma_start(out=pre_vt[0], in_=vsa_d[0][:, 0])

        identF = consts.tile([P, P], f32, name="identF", tag="identF")
        nc.sync.dma_start(out=identF, in_=identF_d)
        identB = consts.tile([P, P], bf16, name="identB", tag="identB")
        nc.sync.dma_start(out=identB, in_=identB_d)
        obig = consts.tile([P, 124], bf16, name="obig", tag="obig")
        nc.sync.dma_start(out=obig, in_=obig_d)
        bmask4 = consts.tile([P, 4], bf16, name="bmask4", tag="bmask4")
        nc.sync.dma_start(out=bmask4, in_=bmask4_d)
        bm4 = consts.tile([P, D], f32, name="bm4", tag="bm4")
        nc.sync.dma_start(out=bm4, in_=bm4_d)
        qm = consts.tile([P, 4, DH], bf16, name="qm", tag="qm")
        nc.sync.dma_start(out=qm, in_=qm_d)
        pm4 = consts.tile([P, 8, 4], bf16, name="pm4", tag="pm4")
        nc.sync.dma_start(out=pm4, in_=pm4_d)

        Wt, bt, bft = {}, {}, {}
        for n in WNAMES:
            Wt[n] = pwts.tile([P, 4, D], bf16, name="W_" + n, tag="W")
            eng = nc.sync if n == "Wq_sa" else nc.gpsimd
            eng.dma_start(out=Wt[n], in_=W_d[n])
            if use_bias[n]:
                bt[n] = consts.tile([P, 4], f32, name="b_" + n, tag="b_" + n)
                nc.sync.dma_start(out=bt[n], in_=b_d[n])
                if n in BM_BIAS:
                    bft[n] = consts.tile([BC, D], f32, name="bf_" + n, tag="bf_" + n)
                    nc.sync.dma_start(out=bft[n], in_=bcast_row(bf_d[n], 0, BC))

        lng, lnb = [None] * 3, [None] * 3
        for i in range(3):
            if ln_affine[i]:
                lng[i] = consts.tile([BC, D], f32, name=f"lng{i}", tag=f"lng{i}")
                nc.sync.dma_start(out=lng[i], in_=bcast_row(lnp_d, 2 * i, BC))
                lnb[i] = consts.tile([BC, D], f32, name=f"lnb{i}", tag=f"lnb{i}")
                nc.sync.dma_start(out=lnb[i], in_=bcast_row(lnp_d, 2 * i + 1, BC))

        # second l1 group: prefetch behind the consts
        pre_kt[1] = pk1.tile([P, 4, 4, 260], bf16, name="kt1", tag="kt1")
        nc.sync.dma_start(out=pre_kt[1], in_=ksa_d[0][:, 1])
        pre_vt[1] = pv1.tile([P, 4, 2, D], bf16, name="vt1", tag="vt1")
        nc.sync.dma_start(out=pre_vt[1], in_=vsa_d[0][:, 1])

        # ------------------------------------------------------------------
        # helpers
        # ------------------------------------------------------------------
        def proj_dmajor(dst, wname, src_T, relu=False):
            """dst[:, mc, :] (d-major [128, 4, 64] bf16) = act(x @ W + b)."""
            for mc in range(4):
                ps = ppj.tile([P, BC], f32, name="pp_" + wname, tag="pj")
                for kc in range(4):
                    nc.tensor.matmul(
                        ps,
                        lhsT=Wt[wname][:, kc, mc * P:(mc + 1) * P],
                        rhs=src_T[:, kc, :],
                        start=(kc == 0), stop=(kc == 3),
                    )
                bias = bt[wname][:, mc:mc + 1] if use_bias[wname] else 0.0
                func = AF.Relu if relu else AF.Identity
                nc.scalar.activation(dst[:, mc, :], ps, func, bias=bias, scale=1.0)

        def mm_batchmajor(ps, src_T, wname):
            """ps [64, 512] = x @ W   (lhsT = x^T chunks, W as moving)."""
            for kc in range(4):
                nc.tensor.matmul(
                    ps,
                    lhsT=src_T[:, kc, :],
                    rhs=Wt[wname][:, kc, :],
                    start=(kc == 0), stop=(kc == 3),
                )

        def transpose_bm(dst_T, src_bm):
            """[64, 512] f32 batch-major -> d-major [128, 4, 64] bf16."""
            for c in range(4):
                ps = pswt.tile([P, BC], f32, name="ptr", tag="ps_wt")
                nc.tensor.transpose(ps, src_bm[:, c * P:(c + 1) * P],
                                    identF[0:BC, 0:BC])
                nc.vector.tensor_copy(dst_T[:, c, :], ps)

        def layer_norm(dst_bm, t_bm, idx):
            stats = small.tile([BC, 6], f32, name="stats", tag="stats")
            nc.vector.bn_stats(stats, t_bm)
            mv = small.tile([BC, 2], f32, name="mv", tag="mv")
            nc.vector.bn_aggr(mv, stats)
            sd = small.tile([BC, 1], f32, name="sd", tag="sd")
            nc.scalar.activation(sd, mv[:, 1:2], AF.Sqrt, bias=eps[0:BC], scale=1.0)
            rstd = small.tile([BC, 1], f32, name="rstd", tag="rstd")
            nc.vector.reciprocal(rstd, sd)
            nmr = small.tile([BC, 1], f32, name="nmr", tag="nmr")
            nc.vector.scalar_tensor_tensor(out=nmr, in0=mv[:, 0:1], scalar=-1.0,
                                           in1=rstd, op0=ALU.mult, op1=ALU.mult)
            if ln_affine[idx]:
                xn = big_tmp.tile([BC, D], f32, name="xn", tag="xn")
                nc.scalar.activation(xn, t_bm, AF.Identity, bias=nmr, scale=rstd)
                nc.vector.tensor_mul(xn, xn, lng[idx])
                nc.vector.tensor_add(dst_bm, xn, lnb[idx])
            else:
                nc.scalar.activation(dst_bm, t_bm, AF.Identity, bias=nmr, scale=rstd)

        def residual_ln(dst_bm, dst_T, src_T, wname, x_prev_bm, idx):
            """dst = LN(x_prev + src @ W + b); also produce d-major dst_T."""
            ps = ppj.tile([BC, D], f32, name="pr_" + wname, tag="pj")
            mm_batchmajor(ps, src_T, wname)
            t_bm = big_tmp.tile([BC, D], f32, name="t_bm", tag="t_bm")
            nc.vector.tensor_add(t_bm, ps, x_prev_bm)
            if use_bias[wname]:
                nc.vector.tensor_add(t_bm, t_bm, bft[wname])
            layer_norm(dst_bm, t_bm, idx)
            transpose_bm(dst_T, dst_bm)

        def build_qbd(q_T, g):
            """[128, 4(j), 4(c), 32] bf16: sample 4g+j's q placed per-head."""
            qbd = pqbd.tile([P, 4, 4, DH], bf16, name="qbd", tag="qbd")
            in0 = q_T[:, :, 4 * g:4 * g + 4].transpose([0, 2, 1]) \
                .unsqueeze(3).broadcast_to([P, 4, 4, DH])
            in1 = qm.unsqueeze(1).broadcast_to([P, 4, 4, DH])
            nc.vector.tensor_mul(qbd, in0, in1)
            return qbd

        # ------------------------------------------------------------------
        # projections from x0 = h_t
        # ------------------------------------------------------------------
        q_saT = acts.tile([P, 4, BC], bf16, name="q_saT", tag="q_saT")
        proj_dmajor(q_saT, "Wq_sa", x0T)
        k_saT = acts.tile([P, 4, BC], bf16, name="k_saT", tag="k_saT")
        proj_dmajor(k_saT, "Wk_sa", x0T)

        v_bmb = acts.tile([BC, D], bf16, name="v_bmb", tag="v_bmb")
        psv = ppj.tile([BC, D], f32, name="psv", tag="pj")
        mm_batchmajor(psv, x0T, "Wv_sa")
        if use_bias["Wv_sa"]:
            tv = big_tmp.tile([BC, D], f32, name="tv", tag="tv")
            nc.vector.tensor_add(tv, psv, bft["Wv_sa"])
            nc.vector.tensor_copy(v_bmb, tv)
        else:
            nc.scalar.copy(v_bmb, psv)

        # ------------------------------------------------------------------
        # layer 1: self-attention over (K_sa | k_sa)
        # ------------------------------------------------------------------
        wcols = T + 1
        ps_at1 = pacc.tile([BC, D], f32, name="ps_at1", tag="pj")
        for g in range(NG):
            if g in pre_kt:
                kt, vt = pre_kt[g], pre_vt[g]
            else:
                # 260-col rows (host zero-padded): contiguous dma, 8B-aligned
                kt = pk1.tile([P, 4, 4, 260], bf16, name="kt1", tag="kt1")
                nc.sync.dma_start(out=kt, in_=ksa_d[g // 2][:, g % 2])
                vt = pv1.tile([P, 4, 2, D], bf16, name="vt1", tag="vt1")
                nc.sync.dma_start(out=vt, in_=vsa_d[g // 2][:, g % 2])
            # new-token key column (k_saT is [P, 4(c), BC])
            nc.vector.tensor_copy(
                kt[:, :, :, T:T + 1],
                k_saT[:, :, 4 * g:4 * g + 4].transpose([0, 2, 1]).unsqueeze(3))
            qbd = build_qbd(q_saT, g)
            vr4 = pvr.tile([4, D], bf16, name="vr4", tag="vr")
            nc.gpsimd.dma_start(out=vr4, in_=v_bmb[4 * g:4 * g + 4, :])

            ps_sc = psc.tile([P, wcols], f32, name="ps_sc", tag="ps_sc")
            for j in range(4):
                for c in range(4):
                    nc.tensor.matmul(
                        ps_sc[32 * j:32 * j + 32, :],
                        lhsT=qbd[:, j, c, :],
                        rhs=kt[:, j, c, 0:wcols],
                        start=(c == 0), stop=(c == 3),
                        tile_position=(0, 32 * j))
            esc = pesc.tile([P, wcols], bf16, name="esc", tag="esc")
            sumexp = small.tile([P, 1], f32, name="sumexp", tag="sumexp")
            nc.scalar.activation(esc, ps_sc, AF.Exp, bias=0.0, scale=1.0,
                                 accum_out=sumexp)
            recip = small.tile([P, 1], f32, name="recip", tag="recip")
            nc.vector.reciprocal(recip, sumexp)
            # new-token softmax weights, banded: escb4[p, r] =
            # esc[p, 256] * (p//32 == r); transposed it becomes the K=4
            # block-diagonal lhsT for the new-token value product.
            escb4 = pesc.tile([P, 4], bf16, name="escb4", tag="escb4")
            nc.vector.tensor_mul(escb4, esc[:, T:T + 1].broadcast_to([P, 4]),
                                 bmask4)

            ps_wt = pswt.tile([P, 3, P], bf16, name="ps_wt", tag="ps_wt")
            wt = pwt.tile([P, 3, P], bf16, name="wt", tag="wt")
            for c, cw, src in [(0, P, esc[:, 0:P]), (1, P, esc[:, P:2 * P]),
                               (2, 4, escb4)]:
                nc.tensor.transpose(ps_wt[0:cw, c, :], src, identB)
                nc.vector.tensor_copy(wt[0:cw, c, :], ps_wt[0:cw, c, :])

            ps_pt = pspt.tile([P, D], f32, name="ps_pt", tag="ps_pt")
            for j in range(4):
                for kc in range(2):
                    nc.tensor.matmul(
                        ps_pt[32 * j:32 * j + 32, :],
                        lhsT=wt[:, kc, 32 * j:32 * j + 32],
                        rhs=vt[:, j, kc, :],
                        start=(kc == 0), stop=False,
                        tile_position=(0, 32 * j), skip_group_check=True)
            # new-token contribution last, so the band matmuls above never
            # wait on the (small, late) vr4 staging dma
            nc.tensor.matmul(ps_pt, lhsT=wt[0:4, 2, :], rhs=vr4,
                             start=False, stop=True, skip_group_check=True)
            ex = pex.tile([P, D], bf16, name="ex", tag="ex")
            nc.vector.scalar_tensor_tensor(
                out=ex, in0=ps_pt, scalar=recip, in1=bm4,
                op0=ALU.mult, op1=ALU.mult)
            nc.tensor.matmul(ps_at1, lhsT=obig[:, 60 - 4 * g:124 - 4 * g],
                             rhs=ex, start=(g == 0), stop=(g == NG - 1))

        attn1 = acts.tile([BC, D], f32, name="attn1", tag="attn1")
        nc.scalar.copy(attn1, ps_at1)
        x1_bm = acts.tile([BC, D], f32, name="x1_bm", tag="x1_bm")
        x1T = acts.tile([P, 4, BC], bf16, name="x1T", tag="x1T")
        attn1T = acts.tile([P, 4, BC], bf16, name="attn1T", tag="attn1T")
        transpose_bm(attn1T, attn1)
        residual_ln(x1_bm, x1T, attn1T, "W0_sa", h_bm, 0)

        # ------------------------------------------------------------------
        # layer 2: encoder attention (masked, padded keys)
        # ------------------------------------------------------------------
        q_aT = acts.tile([P, 4, BC], bf16, name="q_aT", tag="q_aT")
        proj_dmajor(q_aT, "Wq_a", x1T)

        ps_at2 = pacc.tile([BC, D], f32, name="ps_at2", tag="pj")
        for G2 in range(NG2):
            kt = pk2.tile([P, 2, 4, 4, NP], bf16, name="kt2", tag="kt2")
            nc.sync.dma_start(out=kt, in_=ka0_d[G2])
            vt = pv2.tile([P, 2, 4, 2, D], fp8, name="vt2", tag="vt2")
            nc.sync.dma_start(out=vt, in_=va0_d[G2])
            for u in range(2):
                g = 2 * G2 + u
                qbd = build_qbd(q_aT, g)

                mt = pmsk.tile([P, NP], f32, name="mt", tag="m2")
                nc.sync.dma_start(out=mt, in_=mask2_d[g])

                ps_sc = psc.tile([P, NP], f32, name="ps_sc2", tag="ps_sc")
                for j in range(4):
                    for c in range(4):
                        nc.tensor.matmul(
                            ps_sc[32 * j:32 * j + 32, :],
                            lhsT=qbd[:, j, c, :],
                            rhs=kt[:, u, j, c, :],
                            start=(c == 0), stop=(c == 3),
                            tile_position=(0, 32 * j))
                nc.vector.tensor_add(ps_sc, ps_sc, mt)
                esc = pesc.tile([P, NP], bf16, name="esc2", tag="esc")
                sumexp = small.tile([P, 1], f32, name="sumexp2", tag="sumexp")
                nc.scalar.activation(esc, ps_sc, AF.Exp, bias=0.0, scale=1.0,
                                     accum_out=sumexp)
                recip = small.tile([P, 1], f32, name="recip2", tag="recip")
                nc.vector.reciprocal(recip, sumexp)

                ps_wt = pswt.tile([P, 2, P], bf16, name="ps_wt2", tag="ps_wt")
                wt = pwt.tile([P, 2, P], bf16, name="wt2", tag="wt")
                for c in range(2):
                    nc.tensor.transpose(ps_wt[:, c, :],
                                        esc[:, c * P:(c + 1) * P], identB)
                    nc.vector.tensor_copy(wt[:, c, :], ps_wt[:, c, :])

                ps_pt = pspt.tile([P, D], f32, name="ps_pt2", tag="ps_pt")
                for j in range(4):
                    for kc in range(2):
                        nc.tensor.matmul(
                            ps_pt[32 * j:32 * j + 32, :],
                            lhsT=wt[:, kc, 32 * j:32 * j + 32],
                            rhs=vt[:, u, j, kc, :],
                            start=(kc == 0), stop=(kc == 1),
                            tile_position=(0, 32 * j))
                ex = pex.tile([P, D], bf16, name="ex2", tag="ex")
                nc.vector.scalar_tensor_tensor(
                    out=ex, in0=ps_pt, scalar=recip, in1=bm4,
                    op0=ALU.mult, op1=ALU.mult)
                nc.tensor.matmul(ps_at2, lhsT=obig[:, 60 - 4 * g:124 - 4 * g],
                                 rhs=ex, start=(g == 0), stop=(g == NG - 1))

        attn2 = acts.tile([BC, D], f32, name="attn2", tag="attn2")
        nc.scalar.copy(attn2, ps_at2)
        x2_bm = acts.tile([BC, D], f32, name="x2_bm", tag="x2_bm")
        x2T = acts.tile([P, 4, BC], bf16, name="x2T", tag="x2T")
        attn2T = acts.tile([P, 4, BC], bf16, name="attn2T", tag="attn2T")
        transpose_bm(attn2T, attn2)
        residual_ln(x2_bm, x2T, attn2T, "W0_a", x1_bm, 1)

        # ------------------------------------------------------------------
        # MLP
        # ------------------------------------------------------------------
        h1T = acts.tile([P, 4, BC], bf16, name="h1T", tag="h1T")
        proj_dmajor(h1T, "W1", x2T, relu=True)
        x3_bm = acts.tile([BC, D], f32, name="x3_bm", tag="x3_bm")
        x3T = acts.tile([P, 4, BC], bf16, name="x3T", tag="x3T")
        residual_ln(x3_bm, x3T, h1T, "W2", x2_bm, 2)

        qfT = acts.tile([P, 4, BC], bf16, name="qfT", tag="qfT")
        proj_dmajor(qfT, "Wqf", x3T)

        # ------------------------------------------------------------------
        # final pointer scores: w = softmax(10*tanh(qf.K/sqrt(D)) + mask)
        # 8 samples per group: rows 32*q4 + r  (q4 in 0..2, r in 0..4)
        # ------------------------------------------------------------------
        for G in range(NGF):
            kf = pkf.tile([P, 8, 4, NP], bf16, name="kf", tag="kf")
            nc.sync.dma_start(out=kf, in_=kaf_d[G])
            # qfb[p, c, s, r] = qfT[p, c, 8G+s] * (s%4 == r)
            qfb = pqbd.tile([P, 4, 8, 4], bf16, name="qfb", tag="qfb")
            in0 = qfT[:, :, 8 * G:8 * G + 8].unsqueeze(3) \
                .broadcast_to([P, 4, 8, 4])
            in1 = pm4.unsqueeze(1).broadcast_to([P, 4, 8, 4])
            nc.vector.tensor_mul(qfb, in0, in1)

            ps_f = psc.tile([BC, NP], f32, name="ps_f", tag="ps_sc")
            nc.vector.memset(ps_f, 0.0)
            for q4 in range(2):
                for r in range(4):
                    for c in range(4):
                        nc.tensor.matmul(
                            ps_f[32 * q4:32 * q4 + 4, :],
                            lhsT=qfb[:, c, 4 * q4 + r, :],
                            rhs=kf[:, 4 * q4 + r, c, :],
                            start=(r == 0 and c == 0), stop=(r == 3 and c == 3),
                            tile_position=(0, 32 * q4))
            mf = pmsk.tile([BC, NP], f32, name="mf", tag="mf")
            nc.sync.dma_start(out=mf, in_=maskF_d[G])
            t1 = pfin.tile([BC, NP], f32, name="t1", tag="t1")
            nc.scalar.activation(t1, ps_f, AF.Tanh, bias=0.0, scale=1.0)
            t2 = pfin.tile([BC, NP], f32, name="t2", tag="t2")
            nc.vector.scalar_tensor_tensor(out=t2, in0=t1, scalar=10.0,
                                           in1=mf,
                                           op0=ALU.mult, op1=ALU.add)
            e = pfin.tile([BC, NP], f32, name="e", tag="e")
            fsum = small.tile([BC, 1], f32, name="fsum", tag="fsum")
            nc.scalar.activation(e, t2, AF.Exp, bias=0.0, scale=1.0,
                                 accum_out=fsum)
            frec = small.tile([BC, 1], f32, name="frec", tag="frec")
            nc.vector.reciprocal(frec, fsum)
            wf = pfin.tile([BC, NK], f32, name="wf", tag="wf")
            nc.vector.tensor_scalar_mul(wf, e[:, 0:NK], frec)
            nc.gpsimd.dma_start(out=out_d[8 * G:8 * G + 4, :], in_=wf[0:4, :])
            nc.gpsimd.dma_start(out=out_d[8 * G + 4:8 * G + 8, :],
                                in_=wf[32:36, :])

    nc.compile()
    return nc


# ----------------------------------------------------------------------------
# host side
# ----------------------------------------------------------------------------

def _get_program(flags):
    if flags not in _cache:
        _cache[flags] = _build_program(flags)
    return _cache[flags]


def _prep_inputs(inputs):
    """Host-side sharding + layout prep; returns (flags, per-core input maps)."""
    f = np.float32
    h_t = np.asarray(inputs["h_t"], f)
    K_att = np.asarray(inputs["K_att"], f)
    V_att = np.asarray(inputs["V_att"], f)
    K_sa = np.asarray(inputs["K_sa"], f)
    V_sa = np.asarray(inputs["V_sa"], f)
    mask = np.asarray(inputs["mask"])

    sc = np.float32(DH ** -0.5)
    scf = np.float32(D ** -0.5)
    W = {n: np.asarray(inputs[n], f) for n in WNAMES}
    W["Wq_sa"] = W["Wq_sa"] * sc
    W["Wq_a"] = W["Wq_a"] * sc
    W["Wqf"] = W["Wqf"] * scf
    bias_src = {"Wq_sa": "bq_sa", "Wk_sa": "bk_sa", "Wv_sa": "bv_sa",
                "W0_sa": "b0_sa", "Wq_a": "bq_a", "W0_a": "b0_a",
                "W1": "b1", "W2": "b2", "Wqf": "bqf"}
    bvec = {n: np.asarray(inputs[bias_src[n]], f).copy() for n in WNAMES}
    bvec["Wq_sa"] *= sc
    bvec["Wq_a"] *= sc
    bvec["Wqf"] *= scf
    use_bias = tuple(bool(np.any(bvec[n])) for n in WNAMES)
    ub = dict(zip(WNAMES, use_bias))

    lnp = np.stack([np.asarray(inputs[k], f) for k in
                    ["ln1_g", "ln1_b", "ln2_g", "ln2_b", "ln3_g", "ln3_b"]])
    ln_affine = tuple(
        bool(np.any(lnp[2 * i] != 1.0) or np.any(lnp[2 * i + 1] != 0.0))
        for i in range(3))
    flags = (use_bias, ln_affine)

    # d-major weight slabs [128, 4, 512] bf16
    Wb = {n: np.ascontiguousarray(
        W[n].reshape(4, P, D).transpose(1, 0, 2)).astype(BF) for n in WNAMES}

    # streams, host-packed per pair of 4-sample groups (final: 8), bf16
    # ksa[core][G2, p, u, j, c, t] = K_sa[64c+8G2+4u+j, t, 128c+p]; keys
    # padded 256 -> 260 so the kt dma is one contiguous run per partition
    ksa = np.zeros((NCORES, NG2, P, 2, 4, 4, 260), BF)
    ksa[..., :T] = (
        K_sa.transpose(0, 2, 1).reshape(NCORES, NG2, 2, 4, 4, P, T)
        .transpose(0, 1, 5, 2, 3, 4, 6)).astype(BF)
    vsa = np.ascontiguousarray(
        V_sa.reshape(NCORES, NG2, 2, 4, 2, P, D)
        .transpose(0, 1, 5, 2, 3, 4, 6)).astype(BF)
    ka0t = np.zeros((B, D, NP), f)
    ka0t[:, :, :NK] = K_att[:, :, :D].transpose(0, 2, 1)
    ka0 = np.ascontiguousarray(
        ka0t.reshape(NCORES, NG2, 2, 4, 4, P, NP)
        .transpose(0, 1, 5, 2, 3, 4, 6)).astype(BF)
    va0p = np.zeros((B, NP, D), f)
    va0p[:, :NK, :] = V_att[:, :, :D]
    va0 = np.ascontiguousarray(
        va0p.reshape(NCORES, NG2, 2, 4, 2, P, D)
        .transpose(0, 1, 5, 2, 3, 4, 6)).astype(ml_dtypes.float8_e4m3)
    kaft = np.zeros((B, D, NP), f)
    kaft[:, :, :NK] = K_att[:, :, D:].transpose(0, 2, 1)
    kaf = np.ascontiguousarray(
        kaft.reshape(NCORES, NGF, 8, 4, P, NP)
        .transpose(0, 1, 4, 2, 3, 5)).astype(BF)

    maskadd = np.full((B, NP), -1e9, f)
    maskadd[:, :NK] = np.where(mask, f(-1e9), f(0.0))
    # mask2[core][g, p, n] = maskadd[64c + 4g + p//32, n]
    mask2 = np.ascontiguousarray(
        np.broadcast_to(maskadd.reshape(NCORES, NG, 4, 1, NP),
                        (NCORES, NG, 4, 32, NP)).reshape(NCORES, NG, P, NP))
    # maskF[core][G, p, n] = maskadd[64c + 8G + 4*(p//32) + min(p%32,3), n]
    p_arr = np.arange(BC)
    samp_idx = (8 * np.arange(NGF)[:, None] + 4 * (p_arr // 32)[None, :]
                + np.minimum(p_arr % 32, 3)[None, :])        # [NGF, 64]
    mc_ = maskadd.reshape(NCORES, BC, NP)
    maskF = np.ascontiguousarray(mc_[:, samp_idx, :])        # [core,NGF,64,NP]

    # constants
    identF = np.eye(P, dtype=f)
    identB = np.eye(P).astype(BF)
    obig = np.zeros((P, 124), f)
    for j in range(4):
        obig[32 * j:32 * j + H, 60 + j] = 1.0
    obig = obig.astype(BF)
    bmask4 = np.zeros((P, 4), f)
    for j in range(4):
        bmask4[32 * j:32 * j + 32, j] = 1.0
    bmask4 = bmask4.astype(BF)
    bm4 = np.zeros((P, D), f)
    for j in range(4):
        for hh in range(H):
            bm4[32 * j + hh, DH * hh:DH * (hh + 1)] = 1.0
    # qm[p, c, m] = 1 iff m == head(128c+p)
    qm = np.zeros((P, 4, DH), f)
    for c in range(4):
        for p in range(P):
            qm[p, c, (c * P + p) // DH] = 1.0
    qm = qm.astype(BF)
    pm4 = np.zeros((P, 8, 4), f)
    for s in range(8):
        pm4[:, s, s % 4] = 1.0
    pm4 = pm4.astype(BF)

    hT = np.ascontiguousarray(
        h_t.reshape(NCORES, BC, 4, P).transpose(0, 3, 2, 1)).astype(BF)

    b_dmaj = {n: np.ascontiguousarray(bvec[n].reshape(4, P).T) for n in WNAMES}

    in_maps = []
    for i in range(NCORES):
        sl = slice(BC * i, BC * (i + 1))
        m = {
            "hT": hT[i],
            "h_bm": np.ascontiguousarray(h_t[sl]),
            "ksa": ksa[i],
            "vsa": vsa[i],
            "ka0": ka0[i],
            "va0": va0[i],
            "kaf": kaf[i],
            "mask2": mask2[i],
            "maskF": maskF[i],
            "identF": identF,
            "identB": identB,
            "obig": obig,
            "bmask4": bmask4,
            "bm4": bm4,
            "qm": qm,
            "pm4": pm4,
        }
        for n in WNAMES:
            m["W_" + n] = Wb[n]
            if ub[n]:
                m["b_" + n] = b_dmaj[n]
                if n in BM_BIAS:
                    m["bf_" + n] = bvec[n].reshape(1, D)
        if any(ln_affine):
            m["lnp"] = lnp
        in_maps.append(m)
    return flags, in_maps


def _run(inputs, trace=False):
    flags, in_maps = _prep_inputs(inputs)
    nc = _get_program(flags)
    kwargs = {}
    if trace:
        kwargs = dict(trace=True, trace_cores=[0])
    res = run_bass_kernel_spmd(nc, in_maps, list(range(NCORES)), **kwargs)
    out = np.concatenate([res.results[i]["out"] for i in range(NCORES)], axis=0)
    return np.ascontiguousarray(out.astype(np.float32)), res


def kernel(**inputs):
    return _run(inputs, trace=False)[0]


def kernel_traced(**inputs):
    return _run(inputs, trace=True)


# revision 91
# speedup vs baseline: 1.2573x; 1.0166x over previous
"""Trainium2 Bass kernel for nn_AttentionModel (pointer-network decode step).

Data-parallel over 8 NeuronCores: batch 512 -> 64 samples/core; weights
replicated.  Per core the kernel streams the per-sample K/V slabs from HBM
once in bf16 (~10.6 MB/group-phase, ~90 MB total) and computes:

  self-attn over (K_sa | k_sa) -> LN -> enc attention (masked) -> LN ->
  MLP -> LN -> single-head tanh-clipped pointer scores -> softmax weights.

v2 (bf16 streaming) layout notes:
  - all K/V streams, weights and matmul activations are bf16; PSUM, LN and
    softmax normalization stay fp32.  Streams are host-packed per group of
    4 samples into contiguous [128, ...] slabs so each dma_start moves ~1MB.
  - the on-device-computed new-token key k_sa is copied into column 256 of
    the streamed K tile, so one matmul accumulation produces all 257 scores.
  - scores/softmax use no max subtraction (scores are bounded by ~±7 for
    this model: q rows are LN outputs times 0.05-scale weights).
  - per-group attention outputs are folded into a single [64, 512] PSUM
    accumulator via a shifting block-diagonal ones matrix (Obig), replacing
    the per-group [4,512] fold + SBUF + DMA round trip.
  - the final single-head layer packs 8 samples per PSUM tile using a
    4x4 one-hot placement mask (pm4) so each 32-row band holds 4 samples.
"""

import numpy as np
import ml_dtypes
from contextlib import ExitStack

import concourse.bass as bass
import concourse.tile as tile
from concourse import bacc, mybir
from concourse.bass_utils import run_bass_kernel_spmd

f32 = mybir.dt.float32
bf16 = mybir.dt.bfloat16
fp8 = mybir.dt.float8e4
AF = mybir.ActivationFunctionType
ALU = mybir.AluOpType
AX = mybir.AxisListType

BF = ml_dtypes.bfloat16

P = 128          # SBUF partitions
NCORES = 8
B = 512          # full batch
BC = B // NCORES # batch per core (64)
D = 512          # model dim
H = 16           # heads
DH = 32          # head dim
NK = 251         # encoder keys (nb_nodes + 1)
NP = 256         # encoder keys padded to 256
T = 256          # self-attn cache length (new token appended on device)
NG = BC // 4     # sample groups of 4 (one [128, n] psum tile each)
NG2 = BC // 8    # stream pair-groups (two groups per ~2MB dma)
NGF = BC // 8    # final-layer groups of 8 samples
WNAMES = ["Wq_sa", "Wk_sa", "Wv_sa", "W0_sa", "Wq_a", "W0_a", "W1", "W2", "Wqf"]
# weight matmuls whose bias is applied on batch-major [64, 512] rows
BM_BIAS = {"Wv_sa", "W0_sa", "W0_a", "W2"}

_cache = {}


# ----------------------------------------------------------------------------
# program builder
# ----------------------------------------------------------------------------

def _build_program(flags):
    """flags = (use_bias tuple aligned with WNAMES, ln_affine tuple of 3)."""
    use_bias = dict(zip(WNAMES, flags[0]))
    ln_affine = flags[1]

    nc = bacc.Bacc("TRN2", target_bir_lowering=False, debug=False)

    def din(name, shape, dt=f32):
        return nc.dram_tensor(name, shape, dt, kind="ExternalInput").ap()

    hT_d = din("hT", [P, 4, BC], bf16)
    hbm_d = din("h_bm", [BC, D])
    ksa_d = din("ksa", [NG2, P, 2, 4, 4, 260], bf16)
    vsa_d = din("vsa", [NG2, P, 2, 4, 2, D], bf16)
    ka0_d = din("ka0", [NG2, P, 2, 4, 4, NP], bf16)
    va0_d = din("va0", [NG2, P, 2, 4, 2, D], fp8)
    kaf_d = din("kaf", [NGF, P, 8, 4, NP], bf16)
    mask2_d = din("mask2", [NG, P, NP])
    maskF_d = din("maskF", [NGF, BC, NP])
    W_d = {n: din("W_" + n, [P, 4, D], bf16) for n in WNAMES}
    b_d = {n: din("b_" + n, [P, 4]) for n in WNAMES if use_bias[n]}
    bf_d = {n: din("bf_" + n, [1, D]) for n in WNAMES
            if use_bias[n] and n in BM_BIAS}
    if any(ln_affine):
        lnp_d = din("lnp", [6, D])
    identF_d = din("identF", [P, P])
    identB_d = din("identB", [P, P], bf16)
    obig_d = din("obig", [P, 124], bf16)
    bmask4_d = din("bmask4", [P, 4], bf16)
    bm4_d = din("bm4", [P, D])
    qm_d = din("qm", [P, 4, DH], bf16)
    pm4_d = din("pm4", [P, 8, 4], bf16)

    out_d = nc.dram_tensor("out", [BC, NK], f32, kind="ExternalOutput").ap()

    def bcast_row(ap2d, i, n):
        row = ap2d[i:i + 1, :]
        return bass.AP(tensor=row.tensor, offset=row.offset,
                       ap=[[0, n]] + list(row.ap)[1:])

    with tile.TileContext(nc) as tc, ExitStack() as ctx:
        consts = ctx.enter_context(tc.tile_pool(name="consts", bufs=1))
        pwts = ctx.enter_context(tc.tile_pool(name="wts", bufs=4))
        acts = ctx.enter_context(tc.tile_pool(name="acts", bufs=1))
        small = ctx.enter_context(tc.tile_pool(name="small", bufs=8))
        big_tmp = ctx.enter_context(tc.tile_pool(name="big_tmp", bufs=1))
        # stream pools: co-resident so cross-phase DMA prefetch never blocks
        pk1 = ctx.enter_context(tc.tile_pool(name="l1k", bufs=3))
        pv1 = ctx.enter_context(tc.tile_pool(name="l1v", bufs=2))
        pk2 = ctx.enter_context(tc.tile_pool(name="l2k", bufs=2))
        pv2 = ctx.enter_context(tc.tile_pool(name="l2v", bufs=2))
        pkf = ctx.enter_context(tc.tile_pool(name="fk", bufs=3))
        # attention scratch
        pqbd = ctx.enter_context(tc.tile_pool(name="qbd", bufs=2))
        pesc = ctx.enter_context(tc.tile_pool(name="esc", bufs=2))
        pwt = ctx.enter_context(tc.tile_pool(name="wt", bufs=2))
        pex = ctx.enter_context(tc.tile_pool(name="ex", bufs=2))
        pvr = ctx.enter_context(tc.tile_pool(name="vr", bufs=2))
        pfin = ctx.enter_context(tc.tile_pool(name="fin", bufs=2))
        pmsk = ctx.enter_context(tc.tile_pool(name="msk", bufs=2))
        # PSUM pools -- every distinct tag costs bufs x 1 bank; 8 banks total:
        #   ps_sc(1) + ps_wt(2) + ps_pt(2) + ps_at(1) + pj(2) = 8
        psc = ctx.enter_context(tc.tile_pool(name="psc", bufs=2, space="PSUM"))
        pswt = ctx.enter_context(tc.tile_pool(name="pswt", bufs=2, space="PSUM"))
        pspt = ctx.enter_context(tc.tile_pool(name="pspt", bufs=2, space="PSUM"))
        ppj = ctx.enter_context(tc.tile_pool(name="ppj", bufs=2, space="PSUM"))
        pacc = ppj

        # ------------------------------------------------------------------
        # constants / weights (x0/h first: the initial projections need them)
        # ------------------------------------------------------------------
        x0T = acts.tile([P, 4, BC], bf16, name="x0T", tag="x0T")
        nc.sync.dma_start(out=x0T, in_=hT_d)
        h_bm = acts.tile([BC, D], f32, name="h_bm", tag="h_bm")
        nc.sync.dma_start(out=h_bm, in_=hbm_d)
        # touch every ACT function we use so tables load during startup
        eps = consts.tile([P, 1], f32, name="eps", tag="eps")
        nc.vector.memset(eps, 1e-5)
        warm = consts.tile([P, 1], f32, name="warm", tag="warm")
        for fn in (AF.Exp, AF.Tanh, AF.Sqrt, AF.Identity, AF.Relu):
            nc.scalar.activation(warm, eps, fn, bias=0.0, scale=1.0)

        # pre-issue the first self-attn stream tiles so they transfer while
        # the startup consts/projections run
        pre_kt, pre_vt = {}, {}
        pre_kt[0] = pk1.tile([P, 4, 4, 260], bf16, name="kt1", tag="kt1")
        nc.sync.dma_start(out=pre_kt[0], in_=ksa_d[0][:, 0])
        pre_vt[0] = pv1.tile([P, 4, 2, D], bf16, name="vt1", tag="vt1")
        nc.sync.dma_start(out=pre_vt[0], in_=vsa_d[0][:, 0])

        identF = consts.tile([P, P], f32, name="identF", tag="identF")
        nc.sync.dma_start(out=identF, in_=identF_d)
        identB = consts.tile([P, P], bf16, name="identB", tag="identB")
        nc.sync.dma_start(out=identB, in_=identB_d)
        obig = consts.tile([P, 124], bf16, name="obig", tag="obig")
        nc.sync.dma_start(out=obig, in_=obig_d)
        bmask4 = consts.tile([P, 4], bf16, name="bmask4", tag="bmask4")
        nc.sync.dma_start(out=bmask4, in_=bmask4_d)
        bm4 = consts.tile([P, D], f32, name="bm4", tag="bm4")
        nc.sync.dma_start(out=bm4, in_=bm4_d)
        qm = consts.tile([P, 4, DH], bf16, name="qm", tag="qm")
        nc.sync.dma_start(out=qm, in_=qm_d)
        pm4 = consts.tile([P, 8, 4], bf16, name="pm4", tag="pm4")
        nc.sync.dma_start(out=pm4, in_=pm4_d)

        Wt, bt, bft = {}, {}, {}
        for n in WNAMES:
            Wt[n] = pwts.tile([P, 4, D], bf16, name="W_" + n, tag="W")
            nc.gpsimd.dma_start(out=Wt[n], in_=W_d[n])
            if use_bias[n]:
                bt[n] = consts.tile([P, 4], f32, name="b_" + n, tag="b_" + n)
                nc.sync.dma_start(out=bt[n], in_=b_d[n])
                if n in BM_BIAS:
                    bft[n] = consts.tile([BC, D], f32, name="bf_" + n, tag="bf_" + n)
                    nc.sync.dma_start(out=bft[n], in_=bcast_row(bf_d[n], 0, BC))

        lng, lnb = [None] * 3, [None] * 3
        for i in range(3):
            if ln_affine[i]:
                lng[i] = consts.tile([BC, D], f32, name=f"lng{i}", tag=f"lng{i}")
                nc.sync.dma_start(out=lng[i], in_=bcast_row(lnp_d, 2 * i, BC))
                lnb[i] = consts.tile([BC, D], f32, name=f"lnb{i}", tag=f"lnb{i}")
                nc.sync.dma_start(out=lnb[i], in_=bcast_row(lnp_d, 2 * i + 1, BC))

        # second l1 group: prefetch behind the consts
        pre_kt[1] = pk1.tile([P, 4, 4, 260], bf16, name="kt1", tag="kt1")
        nc.sync.dma_start(out=pre_kt[1], in_=ksa_d[0][:, 1])
        pre_vt[1] = pv1.tile([P, 4, 2, D], bf16, name="vt1", tag="vt1")
        nc.sync.dma_start(out=pre_vt[1], in_=vsa_d[0][:, 1])

        # ------------------------------------------------------------------
        # helpers
        # ------------------------------------------------------------------
        def proj_dmajor(dst, wname, src_T, relu=False):
            """dst[:, mc, :] (d-major [128, 4, 64] bf16) = act(x @ W + b)."""
            for mc in range(4):
                ps = ppj.tile([P, BC], f32, name="pp_" + wname, tag="pj")
                for kc in range(4):
                    nc.tensor.matmul(
                        ps,
                        lhsT=Wt[wname][:, kc, mc * P:(mc + 1) * P],
                        rhs=src_T[:, kc, :],
                        start=(kc == 0), stop=(kc == 3),
                    )
                bias = bt[wname][:, mc:mc + 1] if use_bias[wname] else 0.0
                func = AF.Relu if relu else AF.Identity
                nc.scalar.activation(dst[:, mc, :], ps, func, bias=bias, scale=1.0)

        def mm_batchmajor(ps, src_T, wname):
            """ps [64, 512] = x @ W   (lhsT = x^T chunks, W as moving)."""
            for kc in range(4):
                nc.tensor.matmul(
                    ps,
                    lhsT=src_T[:, kc, :],
                    rhs=Wt[wname][:, kc, :],
                    start=(kc == 0), stop=(kc == 3),
                )

        def transpose_bm(dst_T, src_bm):
            """[64, 512] f32 batch-major -> d-major [128, 4, 64] bf16."""
            for c in range(4):
                ps = pswt.tile([P, BC], f32, name="ptr", tag="ps_wt")
                nc.tensor.transpose(ps, src_bm[:, c * P:(c + 1) * P],
                                    identF[0:BC, 0:BC])
                nc.vector.tensor_copy(dst_T[:, c, :], ps)

        def layer_norm(dst_bm, t_bm, idx):
            stats = small.tile([BC, 6], f32, name="stats", tag="stats")
            nc.vector.bn_stats(stats, t_bm)
            mv = small.tile([BC, 2], f32, name="mv", tag="mv")
            nc.vector.bn_aggr(mv, stats)
            sd = small.tile([BC, 1], f32, name="sd", tag="sd")
            nc.scalar.activation(sd, mv[:, 1:2], AF.Sqrt, bias=eps[0:BC], scale=1.0)
            rstd = small.tile([BC, 1], f32, name="rstd", tag="rstd")
            nc.vector.reciprocal(rstd, sd)
            nmr = small.tile([BC, 1], f32, name="nmr", tag="nmr")
            nc.vector.scalar_tensor_tensor(out=nmr, in0=mv[:, 0:1], scalar=-1.0,
                                           in1=rstd, op0=ALU.mult, op1=ALU.mult)
            if ln_affine[idx]:
                xn = big_tmp.tile([BC, D], f32, name="xn", tag="xn")
                nc.scalar.activation(xn, t_bm, AF.Identity, bias=nmr, scale=rstd)
                nc.vector.tensor_mul(xn, xn, lng[idx])
                nc.vector.tensor_add(dst_bm, xn, lnb[idx])
            else:
                nc.scalar.activation(dst_bm, t_bm, AF.Identity, bias=nmr, scale=rstd)

        def residual_ln(dst_bm, dst_T, src_T, wname, x_prev_bm, idx):
            """dst = LN(x_prev + src @ W + b); also produce d-major dst_T."""
            ps = ppj.tile([BC, D], f32, name="pr_" + wname, tag="pj")
            mm_batchmajor(ps, src_T, wname)
            t_bm = big_tmp.tile([BC, D], f32, name="t_bm", tag="t_bm")
            nc.vector.tensor_add(t_bm, ps, x_prev_bm)
            if use_bias[wname]:
                nc.vector.tensor_add(t_bm, t_bm, bft[wname])
            layer_norm(dst_bm, t_bm, idx)
            transpose_bm(dst_T, dst_bm)

        def build_qbd(q_T, g):
            """[128, 4(j), 4(c), 32] bf16: sample 4g+j's q placed per-head."""
            qbd = pqbd.tile([P, 4, 4, DH], bf16, name="qbd", tag="qbd")
            in0 = q_T[:, :, 4 * g:4 * g + 4].transpose([0, 2, 1]) \
                .unsqueeze(3).broadcast_to([P, 4, 4, DH])
            in1 = qm.unsqueeze(1).broadcast_to([P, 4, 4, DH])
            nc.vector.tensor_mul(qbd, in0, in1)
            return qbd

        # ------------------------------------------------------------------
        # projections from x0 = h_t
        # ------------------------------------------------------------------
        q_saT = acts.tile([P, 4, BC], bf16, name="q_saT", tag="q_saT")
        proj_dmajor(q_saT, "Wq_sa", x0T)
        k_saT = acts.tile([P, 4, BC], bf16, name="k_saT", tag="k_saT")
        proj_dmajor(k_saT, "Wk_sa", x0T)

        v_bmb = acts.tile([BC, D], bf16, name="v_bmb", tag="v_bmb")
        psv = ppj.tile([BC, D], f32, name="psv", tag="pj")
        mm_batchmajor(psv, x0T, "Wv_sa")
        if use_bias["Wv_sa"]:
            tv = big_tmp.tile([BC, D], f32, name="tv", tag="tv")
            nc.vector.tensor_add(tv, psv, bft["Wv_sa"])
            nc.vector.tensor_copy(v_bmb, tv)
        else:
            nc.scalar.copy(v_bmb, psv)

        # ------------------------------------------------------------------
        # layer 1: self-attention over (K_sa | k_sa)
        # ------------------------------------------------------------------
        wcols = T + 1
        ps_at1 = pacc.tile([BC, D], f32, name="ps_at1", tag="pj")
        for g in range(NG):
            if g in pre_kt:
                kt, vt = pre_kt[g], pre_vt[g]
            else:
                # 260-col rows (host zero-padded): contiguous dma, 8B-aligned
                kt = pk1.tile([P, 4, 4, 260], bf16, name="kt1", tag="kt1")
                nc.sync.dma_start(out=kt, in_=ksa_d[g // 2][:, g % 2])
                vt = pv1.tile([P, 4, 2, D], bf16, name="vt1", tag="vt1")
                nc.sync.dma_start(out=vt, in_=vsa_d[g // 2][:, g % 2])
            # new-token key column (k_saT is [P, 4(c), BC])
            nc.vector.tensor_copy(
                kt[:, :, :, T:T + 1],
                k_saT[:, :, 4 * g:4 * g + 4].transpose([0, 2, 1]).unsqueeze(3))
            qbd = build_qbd(q_saT, g)
            vr4 = pvr.tile([4, D], bf16, name="vr4", tag="vr")
            nc.gpsimd.dma_start(out=vr4, in_=v_bmb[4 * g:4 * g + 4, :])

            ps_sc = psc.tile([P, wcols], f32, name="ps_sc", tag="ps_sc")
            for j in range(4):
                for c in range(4):
                    nc.tensor.matmul(
                        ps_sc[32 * j:32 * j + 32, :],
                        lhsT=qbd[:, j, c, :],
                        rhs=kt[:, j, c, 0:wcols],
                        start=(c == 0), stop=(c == 3),
                        tile_position=(0, 32 * j))
            esc = pesc.tile([P, wcols], bf16, name="esc", tag="esc")
            sumexp = small.tile([P, 1], f32, name="sumexp", tag="sumexp")
            nc.scalar.activation(esc, ps_sc, AF.Exp, bias=0.0, scale=1.0,
                                 accum_out=sumexp)
            recip = small.tile([P, 1], f32, name="recip", tag="recip")
            nc.vector.reciprocal(recip, sumexp)
            # new-token softmax weights, banded: escb4[p, r] =
            # esc[p, 256] * (p//32 == r); transposed it becomes the K=4
            # block-diagonal lhsT for the new-token value product.
            escb4 = pesc.tile([P, 4], bf16, name="escb4", tag="escb4")
            nc.vector.tensor_mul(escb4, esc[:, T:T + 1].broadcast_to([P, 4]),
                                 bmask4)

            ps_wt = pswt.tile([P, 3, P], bf16, name="ps_wt", tag="ps_wt")
            wt = pwt.tile([P, 3, P], bf16, name="wt", tag="wt")
            for c, cw, src in [(0, P, esc[:, 0:P]), (1, P, esc[:, P:2 * P]),
                               (2, 4, escb4)]:
                nc.tensor.transpose(ps_wt[0:cw, c, :], src, identB)
                nc.vector.tensor_copy(wt[0:cw, c, :], ps_wt[0:cw, c, :])

            ps_pt = pspt.tile([P, D], f32, name="ps_pt", tag="ps_pt")
            for j in range(4):
                for kc in range(2):
                    nc.tensor.matmul(
                        ps_pt[32 * j:32 * j + 32, :],
                        lhsT=wt[:, kc, 32 * j:32 * j + 32],
                        rhs=vt[:, j, kc, :],
                        start=(kc == 0), stop=False,
                        tile_position=(0, 32 * j), skip_group_check=True)
            # new-token contribution last, so the band matmuls above never
            # wait on the (small, late) vr4 staging dma
            nc.tensor.matmul(ps_pt, lhsT=wt[0:4, 2, :], rhs=vr4,
                             start=False, stop=True, skip_group_check=True)
            ex = pex.tile([P, D], bf16, name="ex", tag="ex")
            nc.vector.scalar_tensor_tensor(
                out=ex, in0=ps_pt, scalar=recip, in1=bm4,
                op0=ALU.mult, op1=ALU.mult)
            nc.tensor.matmul(ps_at1, lhsT=obig[:, 60 - 4 * g:124 - 4 * g],
                             rhs=ex, start=(g == 0), stop=(g == NG - 1))

        attn1 = acts.tile([BC, D], f32, name="attn1", tag="attn1")
        nc.scalar.copy(attn1, ps_at1)
        x1_bm = acts.tile([BC, D], f32, name="x1_bm", tag="x1_bm")
        x1T = acts.tile([P, 4, BC], bf16, name="x1T", tag="x1T")
        attn1T = acts.tile([P, 4, BC], bf16, name="attn1T", tag="attn1T")
        transpose_bm(attn1T, attn1)
        residual_ln(x1_bm, x1T, attn1T, "W0_sa", h_bm, 0)

        # ------------------------------------------------------------------
        # layer 2: encoder attention (masked, padded keys)
        # ------------------------------------------------------------------
        q_aT = acts.tile([P, 4, BC], bf16, name="q_aT", tag="q_aT")
        proj_dmajor(q_aT, "Wq_a", x1T)

        ps_at2 = pacc.tile([BC, D], f32, name="ps_at2", tag="pj")
        for G2 in range(NG2):
            kt = pk2.tile([P, 2, 4, 4, NP], bf16, name="kt2", tag="kt2")
            nc.sync.dma_start(out=kt, in_=ka0_d[G2])
            vt = pv2.tile([P, 2, 4, 2, D], fp8, name="vt2", tag="vt2")
            nc.sync.dma_start(out=vt, in_=va0_d[G2])
            for u in range(2):
                g = 2 * G2 + u
                qbd = build_qbd(q_aT, g)

                mt = pmsk.tile([P, NP], f32, name="mt", tag="m2")
                nc.sync.dma_start(out=mt, in_=mask2_d[g])

                ps_sc = psc.tile([P, NP], f32, name="ps_sc2", tag="ps_sc")
                for j in range(4):
                    for c in range(4):
                        nc.tensor.matmul(
                            ps_sc[32 * j:32 * j + 32, :],
                            lhsT=qbd[:, j, c, :],
                            rhs=kt[:, u, j, c, :],
                            start=(c == 0), stop=(c == 3),
                            tile_position=(0, 32 * j))
                nc.vector.tensor_add(ps_sc, ps_sc, mt)
                esc = pesc.tile([P, NP], bf16, name="esc2", tag="esc")
                sumexp = small.tile([P, 1], f32, name="sumexp2", tag="sumexp")
                nc.scalar.activation(esc, ps_sc, AF.Exp, bias=0.0, scale=1.0,
                                     accum_out=sumexp)
                recip = small.tile([P, 1], f32, name="recip2", tag="recip")
                nc.vector.reciprocal(recip, sumexp)

                ps_wt = pswt.tile([P, 2, P], bf16, name="ps_wt2", tag="ps_wt")
                wt = pwt.tile([P, 2, P], bf16, name="wt2", tag="wt")
                for c in range(2):
                    nc.tensor.transpose(ps_wt[:, c, :],
                                        esc[:, c * P:(c + 1) * P], identB)
                    nc.vector.tensor_copy(wt[:, c, :], ps_wt[:, c, :])

                ps_pt = pspt.tile([P, D], f32, name="ps_pt2", tag="ps_pt")
                for j in range(4):
                    for kc in range(2):
                        nc.tensor.matmul(
                            ps_pt[32 * j:32 * j + 32, :],
                            lhsT=wt[:, kc, 32 * j:32 * j + 32],
                            rhs=vt[:, u, j, kc, :],
                            start=(kc == 0), stop=(kc == 1),
                            tile_position=(0, 32 * j))
                ex = pex.tile([P, D], bf16, name="ex2", tag="ex")
                nc.vector.scalar_tensor_tensor(
                    out=ex, in0=ps_pt, scalar=recip, in1=bm4,
                    op0=ALU.mult, op1=ALU.mult)
                nc.tensor.matmul(ps_at2, lhsT=obig[:, 60 - 4 * g:124 - 4 * g],
                                 rhs=ex, start=(g == 0), stop=(g == NG - 1))

        attn2 = acts.tile([BC, D], f32, name="attn2", tag="attn2")
        nc.scalar.copy(attn2, ps_at2)
        x2_bm = acts.tile([BC, D], f32, name="x2_bm", tag="x2_bm")
        x2T = acts.tile([P, 4, BC], bf16, name="x2T", tag="x2T")
        attn2T = acts.tile([P, 4, BC], bf16, name="attn2T", tag="attn2T")
        transpose_bm(attn2T, attn2)
        residual_ln(x2_bm, x2T, attn2T, "W0_a", x1_bm, 1)

        # ------------------------------------------------------------------
        # MLP
        # ------------------------------------------------------------------
        h1T = acts.tile([P, 4, BC], bf16, name="h1T", tag="h1T")
        proj_dmajor(h1T, "W1", x2T, relu=True)
        x3_bm = acts.tile([BC, D], f32, name="x3_bm", tag="x3_bm")
        x3T = acts.tile([P, 4, BC], bf16, name="x3T", tag="x3T")
        residual_ln(x3_bm, x3T, h1T, "W2", x2_bm, 2)

        qfT = acts.tile([P, 4, BC], bf16, name="qfT", tag="qfT")
        proj_dmajor(qfT, "Wqf", x3T)

        # ------------------------------------------------------------------
        # final pointer scores: w = softmax(10*tanh(qf.K/sqrt(D)) + mask)
        # 8 samples per group: rows 32*q4 + r  (q4 in 0..2, r in 0..4)
        # ------------------------------------------------------------------
        for G in range(NGF):
            kf = pkf.tile([P, 8, 4, NP], bf16, name="kf", tag="kf")
            nc.sync.dma_start(out=kf, in_=kaf_d[G])
            # qfb[p, c, s, r] = qfT[p, c, 8G+s] * (s%4 == r)
            qfb = pqbd.tile([P, 4, 8, 4], bf16, name="qfb", tag="qfb")
            in0 = qfT[:, :, 8 * G:8 * G + 8].unsqueeze(3) \
                .broadcast_to([P, 4, 8, 4])
            in1 = pm4.unsqueeze(1).broadcast_to([P, 4, 8, 4])
            nc.vector.tensor_mul(qfb, in0, in1)

            ps_f = psc.tile([BC, NP], f32, name="ps_f", tag="ps_sc")
            nc.vector.memset(ps_f, 0.0)
            for q4 in range(2):
                for r in range(4):
                    for c in range(4):
                        nc.tensor.matmul(
                            ps_f[32 * q4:32 * q4 + 4, :],
                            lhsT=qfb[:, c, 4 * q4 + r, :],
                            rhs=kf[:, 4 * q4 + r, c, :],
                            start=(r == 0 and c == 0), stop=(r == 3 and c == 3),
                            tile_position=(0, 32 * q4))
            mf = pmsk.tile([BC, NP], f32, name="mf", tag="mf")
            nc.sync.dma_start(out=mf, in_=maskF_d[G])
            t1 = pfin.tile([BC, NP], f32, name="t1", tag="t1")
            nc.scalar.activation(t1, ps_f, AF.Tanh, bias=0.0, scale=1.0)
            t2 = pfin.tile([BC, NP], f32, name="t2", tag="t2")
            nc.vector.scalar_tensor_tensor(out=t2, in0=t1, scalar=10.0,
                                           in1=mf,
                                           op0=ALU.mult, op1=ALU.add)
            e = pfin.tile([BC, NP], f32, name="e", tag="e")
            fsum = small.tile([BC, 1], f32, name="fsum", tag="fsum")
            nc.scalar.activation(e, t2, AF.Exp, bias=0.0, scale=1.0,
                                 accum_out=fsum)
            frec = small.tile([BC, 1], f32, name="frec", tag="frec")
            nc.vector.reciprocal(frec, fsum)
            wf = pfin.tile([BC, NK], f32, name="wf", tag="wf")
            nc.vector.tensor_scalar_mul(wf, e[:, 0:NK], frec)
            nc.gpsimd.dma_start(out=out_d[8 * G:8 * G + 4, :], in_=wf[0:4, :])
            nc.gpsimd.dma_start(out=out_d[8 * G + 4:8 * G + 8, :],
                                in_=wf[32:36, :])

    nc.compile()
    return nc


# ----------------------------------------------------------------------------
# host side
# ----------------------------------------------------------------------------

def _get_program(flags):
    if flags not in _cache:
        _cache[flags] = _build_program(flags)
    return _cache[flags]


def _prep_inputs(inputs):
    """Host-side sharding + layout prep; returns (flags, per-core input maps)."""
    f = np.float32
    h_t = np.asarray(inputs["h_t"], f)
    K_att = np.asarray(inputs["K_att"], f)
    V_att = np.asarray(inputs["V_att"], f)
    K_sa = np.asarray(inputs["K_sa"], f)
    V_sa = np.asarray(inputs["V_sa"], f)
    mask = np.asarray(inputs["mask"])

    sc = np.float32(DH ** -0.5)
    scf = np.float32(D ** -0.5)
    W = {n: np.asarray(inputs[n], f) for n in WNAMES}
    W["Wq_sa"] = W["Wq_sa"] * sc
    W["Wq_a"] = W["Wq_a"] * sc
    W["Wqf"] = W["Wqf"] * scf
    bias_src = {"Wq_sa": "bq_sa", "Wk_sa": "bk_sa", "Wv_sa": "bv_sa",
                "W0_sa": "b0_sa", "Wq_a": "bq_a", "W0_a": "b0_a",
                "W1": "b1", "W2": "b2", "Wqf": "bqf"}
    bvec = {n: np.asarray(inputs[bias_src[n]], f).copy() for n in WNAMES}
    bvec["Wq_sa"] *= sc
    bvec["Wq_a"] *= sc
    bvec["Wqf"] *= scf
    use_bias = tuple(bool(np.any(bvec[n])) for n in WNAMES)
    ub = dict(zip(WNAMES, use_bias))

    lnp = np.stack([np.asarray(inputs[k], f) for k in
                    ["ln1_g", "ln1_b", "ln2_g", "ln2_b", "ln3_g", "ln3_b"]])
    ln_affine = tuple(
        bool(np.any(lnp[2 * i] != 1.0) or np.any(lnp[2 * i + 1] != 0.0))
        for i in range(3))
    flags = (use_bias, ln_affine)

    # d-major weight slabs [128, 4, 512] bf16
    Wb = {n: np.ascontiguousarray(
        W[n].reshape(4, P, D).transpose(1, 0, 2)).astype(BF) for n in WNAMES}

    # streams, host-packed per pair of 4-sample groups (final: 8), bf16
    # ksa[core][G2, p, u, j, c, t] = K_sa[64c+8G2+4u+j, t, 128c+p]; keys
    # padded 256 -> 260 so the kt dma is one contiguous run per partition
    ksa = np.zeros((NCORES, NG2, P, 2, 4, 4, 260), BF)
    ksa[..., :T] = (
        K_sa.transpose(0, 2, 1).reshape(NCORES, NG2, 2, 4, 4, P, T)
        .transpose(0, 1, 5, 2, 3, 4, 6)).astype(BF)
    vsa = np.ascontiguousarray(
        V_sa.reshape(NCORES, NG2, 2, 4, 2, P, D)
        .transpose(0, 1, 5, 2, 3, 4, 6)).astype(BF)
    ka0t = np.zeros((B, D, NP), f)
    ka0t[:, :, :NK] = K_att[:, :, :D].transpose(0, 2, 1)
    ka0 = np.ascontiguousarray(
        ka0t.reshape(NCORES, NG2, 2, 4, 4, P, NP)
        .transpose(0, 1, 5, 2, 3, 4, 6)).astype(BF)
    va0p = np.zeros((B, NP, D), f)
    va0p[:, :NK, :] = V_att[:, :, :D]
    va0 = np.ascontiguousarray(
        va0p.reshape(NCORES, NG2, 2, 4, 2, P, D)
        .transpose(0, 1, 5, 2, 3, 4, 6)).astype(ml_dtypes.float8_e4m3)
    kaft = np.zeros((B, D, NP), f)
    kaft[:, :, :NK] = K_att[:, :, D:].transpose(0, 2, 1)
    kaf = np.ascontiguousarray(
        kaft.reshape(NCORES, NGF, 8, 4, P, NP)
        .transpose(0, 1, 4, 2, 3, 5)).astype(BF)

    maskadd = np.full((B, NP), -1e9, f)
    maskadd[:, :NK] = np.where(mask, f(-1e9), f(0.0))
    # mask2[core][g, p, n] = maskadd[64c + 4g + p//32, n]
    mask2 = np.ascontiguousarray(
        np.broadcast_to(maskadd.reshape(NCORES, NG, 4, 1, NP),
                        (NCORES, NG, 4, 32, NP)).reshape(NCORES, NG, P, NP))
    # maskF[core][G, p, n] = maskadd[64c + 8G + 4*(p//32) + min(p%32,3), n]
    p_arr = np.arange(BC)
    samp_idx = (8 * np.arange(NGF)[:, None] + 4 * (p_arr // 32)[None, :]
                + np.minimum(p_arr % 32, 3)[None, :])        # [NGF, 64]
    mc_ = maskadd.reshape(NCORES, BC, NP)
    maskF = np.ascontiguousarray(mc_[:, samp_idx, :])        # [core,NGF,64,NP]

    # constants
    identF = np.eye(P, dtype=f)
    identB = np.eye(P).astype(BF)
    obig = np.zeros((P, 124), f)
    for j in range(4):
        obig[32 * j:32 * j + H, 60 + j] = 1.0
    obig = obig.astype(BF)
    bmask4 = np.zeros((P, 4), f)
    for j in range(4):
        bmask4[32 * j:32 * j + 32, j] = 1.0
    bmask4 = bmask4.astype(BF)
    bm4 = np.zeros((P, D), f)
    for j in range(4):
        for hh in range(H):
            bm4[32 * j + hh, DH * hh:DH * (hh + 1)] = 1.0
    # qm[p, c, m] = 1 iff m == head(128c+p)
    qm = np.zeros((P, 4, DH), f)
    for c in range(4):
        for p in range(P):
            qm[p, c, (c * P + p) // DH] = 1.0
    qm = qm.astype(BF)
    pm4 = np.zeros((P, 8, 4), f)
    for s in range(8):
        pm4[:, s, s % 4] = 1.0
    pm4 = pm4.astype(BF)

    hT = np.ascontiguousarray(
        h_t.reshape(NCORES, BC, 4, P).transpose(0, 3, 2, 1)).astype(BF)

    b_dmaj = {n: np.ascontiguousarray(bvec[n].reshape(4, P).T) for n in WNAMES}

    in_maps = []
    for i in range(NCORES):
        sl = slice(BC * i, BC * (i + 1))
        m = {
            "hT": hT[i],
            "h_bm": np.ascontiguousarray(h_t[sl]),
            "ksa": ksa[i],
            "vsa": vsa[i],
            "ka0": ka0[i],
            "va0": va0[i],
            "kaf": kaf[i],
            "mask2": mask2[i],
            "maskF": maskF[i],
            "identF": identF,
            "identB": identB,
            "obig": obig,
            "bmask4": bmask4,
            "bm4": bm4,
            "qm": qm,
            "pm4": pm4,
        }
        for n in WNAMES:
            m["W_" + n] = Wb[n]
            if ub[n]:
                m["b_" + n] = b_dmaj[n]
                if n in BM_BIAS:
                    m["bf_" + n] = bvec[n].reshape(1, D)
        if any(ln_affine):
            m["lnp"] = lnp
        in_maps.append(m)
    return flags, in_maps


def _run(inputs, trace=False):
    flags, in_maps = _prep_inputs(inputs)
    nc = _get_program(flags)
    kwargs = {}
    if trace:
        kwargs = dict(trace=True, trace_cores=[0])
    res = run_bass_kernel_spmd(nc, in_maps, list(range(NCORES)), **kwargs)
    out = np.concatenate([res.results[i]["out"] for i in range(NCORES)], axis=0)
    return np.ascontiguousarray(out.astype(np.float32)), res


def kernel(**inputs):
    return _run(inputs, trace=False)[0]


def kernel_traced(**inputs):
    return _run(inputs, trace=True)
